# revision 1
# baseline (speedup 1.0000x reference)
"""Trainium2 Bass kernel for ContentAdaptiveSparsity (topk_masking).

Reference semantics (verified numerically): combined[b,i,j,h] =
q_imp[b,i,h] * k_imp[b,j,h] * interaction[b,i,j,h] built from block-mean
pooled q,k (64 blocks of 128) through tiny MLPs.  The reference then does a
RAW row-major reshape of combined [B,nb,nb,H] -> [B,16,4096]: top-k row
r = i//4 mixes all 16 heads, candidate m = (i%4)*1024 + j*16 + h, and the
top-1024 mask is scattered to out[b, r, m//64, m%64].

Sharding: 64 (b,r) rows over 8 cores -> core c handles batch b=c//2 and
rows r in [8*(c%2), 8*(c%2)+8), i.e. i-blocks [32*(c%2), +32).  q slice is
1/8 of q; k[b] is needed in full by both cores of a batch (2x over-read).
Host pre-transposes both to per-head-contiguous [16, S_slice, 128].

Device pipeline per core (grp = 4 heads, 4 grps):
  - pooling per (tensor, head): contiguous 2/4MB DMA -> [128p, F] tile,
    block-diag ones matmuls accumulate 4 partial block sums in PSUM,
    DVE grouped reduce, then a matmul against I/128 transposes+scales
    into qTall [128d, (h,i)=512] / kTall [128d, (h,j)=1024].
  - MLPs at N=128/256 per grp; sigmoid = ACT exp(-x) then 1/(1+e) on DVE
    (accurate, tracks the fp32 reference); interaction grid h via
    broadcast-AP add + relu; block-diag w2 matmul -> [4hh, (i,j)] exp ->
    sigma -> multiply q_imp/k_imp factors (broadcast APs).
  - fold to bisection layout: per-head DMAs into estage3 [32i, (hh,j)],
    DVE free-dim transpose -> estage4 [32i, (j,hh)], then one DMA per r
    -> folded [128, (r,32)] where p = (i%4)*32 + j//2, l = (j%2)*16 + h.
  - top-k: 32-iter threshold bisection, all 8 rows jointly: DVE compare +
    grouped reduce, all-ones matmul replicates counts across partitions,
    partition-local lo/hi/mid update.  Mask = (v >= lo) as uint8.
"""

import numpy as np

B, S, H, D = 4, 8192, 16, 128
NB = 64           # blocks per sequence
NROW = 8          # topk rows (r) per core
NCORES = 8
KSEL = 1024
HID1, HID2 = 32, 16
NITER = 32

_nc_cache = {}


def _build_nc():
    from contextlib import ExitStack

    from concourse import bacc
    import concourse.mybir as mybir
    from concourse.tile import TileContext

    f32 = mybir.dt.float32
    u8 = mybir.dt.uint8
    AF = mybir.ActivationFunctionType
    OP = mybir.AluOpType
    AX = mybir.AxisListType

    nc = bacc.Bacc("TRN2", target_bir_lowering=False, debug=False,
                   num_devices=NCORES)

    qs = nc.dram_tensor("qs", [H, S // 2, D], f32, kind="ExternalInput")
    ks = nc.dram_tensor("ks", [H, S, D], f32, kind="ExternalInput")
    c_blkq = nc.dram_tensor("c_blkq", [128, 32], f32, kind="ExternalInput")
    c_blkk = nc.dram_tensor("c_blkk", [128, 64], f32, kind="ExternalInput")
    c_id32 = nc.dram_tensor("c_id32", [32, 32], f32, kind="ExternalInput")
    c_id64 = nc.dram_tensor("c_id64", [64, 64], f32, kind="ExternalInput")
    c_ones = nc.dram_tensor("c_ones", [128, 128], f32, kind="ExternalInput")
    c_w1 = nc.dram_tensor("c_w1", [D, HID1], f32, kind="ExternalInput")
    c_b1 = nc.dram_tensor("c_b1", [HID1, 1], f32, kind="ExternalInput")
    c_w2 = nc.dram_tensor("c_w2", [HID1, HID2], f32, kind="ExternalInput")
    c_b2 = nc.dram_tensor("c_b2", [HID2, 1], f32, kind="ExternalInput")
    c_w3 = nc.dram_tensor("c_w3", [HID2, 1], f32, kind="ExternalInput")
    c_nb3 = nc.dram_tensor("c_nb3", [1, 1], f32, kind="ExternalInput")
    c_w1a = nc.dram_tensor("c_w1a", [D, HID1], f32, kind="ExternalInput")
    c_w1b = nc.dram_tensor("c_w1b", [D, HID1], f32, kind="ExternalInput")
    c_b1i = nc.dram_tensor("c_b1i", [128, 1], f32, kind="ExternalInput")
    c_w2bd = nc.dram_tensor("c_w2bd", [128, 4], f32, kind="ExternalInput")
    c_nb2i = nc.dram_tensor("c_nb2i", [4, 1], f32, kind="ExternalInput")

    y = nc.dram_tensor("y", [NROW, NB, NB], u8, kind="ExternalOutput")

    with TileContext(nc) as tc, ExitStack() as ctx:
        const = ctx.enter_context(tc.tile_pool(name="const", bufs=1))
        bigk = ctx.enter_context(tc.tile_pool(name="bigk", bufs=3))
        bigq = ctx.enter_context(tc.tile_pool(name="bigq", bufs=2))
        hpool = ctx.enter_context(tc.tile_pool(name="hpool", bufs=2))
        sb = ctx.enter_context(tc.tile_pool(name="sb", bufs=2))
        persist = ctx.enter_context(tc.tile_pool(name="persist", bufs=1))
        pool_ps = ctx.enter_context(tc.tile_pool(name="pool_ps", bufs=2, space="PSUM"))
        tp_ps = ctx.enter_context(tc.tile_pool(name="tp_ps", bufs=2, space="PSUM"))
        small_ps = ctx.enter_context(tc.tile_pool(name="small_ps", bufs=2, space="PSUM"))
        int_ps = ctx.enter_context(tc.tile_pool(name="int_ps", bufs=2, space="PSUM"))

        def cload(dram, shape, tag):
            t = const.tile(shape, f32, tag=tag)
            nc.sync.dma_start(t[:], dram[:])
            return t

        blkq = cload(c_blkq, [128, 32], "blkq")
        blkk = cload(c_blkk, [128, 64], "blkk")
        id32 = cload(c_id32, [32, 32], "id32")
        id64 = cload(c_id64, [64, 64], "id64")
        ones = cload(c_ones, [128, 128], "ones")
        w1 = cload(c_w1, [D, HID1], "w1")
        b1 = cload(c_b1, [HID1, 1], "b1")
        w2 = cload(c_w2, [HID1, HID2], "w2")
        b2 = cload(c_b2, [HID2, 1], "b2")
        w3 = cload(c_w3, [HID2, 1], "w3")
        nb3 = cload(c_nb3, [1, 1], "nb3")
        w1a = cload(c_w1a, [D, HID1], "w1a")
        w1b = cload(c_w1b, [D, HID1], "w1b")
        b1i = cload(c_b1i, [128, 1], "b1i")
        w2bd = cload(c_w2bd, [128, 4], "w2bd")
        nb2i = cload(c_nb2i, [4, 1], "nb2i")

        qTall = persist.tile([128, H * 32], f32, tag="qTall")   # (h, i)
        kTall = persist.tile([128, H * 64], f32, tag="kTall")   # (h, j)
        qimp = persist.tile([1, H * 32], f32, tag="qimp")
        kimp = persist.tile([1, H * 64], f32, tag="kimp")
        estage3 = persist.tile([32, H * 64], f32, tag="estage3")  # (hh, j)
        estage4 = persist.tile([32, H * 64], f32, tag="estage4")  # (j, hh)
        folded = persist.tile([128, NROW * 32], f32, tag="folded")

        def pool_q(h):
            xt = bigq.tile([128, 4096], f32, tag="xtq")
            nc.sync.dma_start(xt[:], qs[h].rearrange("(a b) d -> a (b d)", a=128))
            ps = pool_ps.tile([32, 512], f32, tag="pool")
            for j in range(8):
                nc.tensor.matmul(ps[:], lhsT=blkq[:], rhs=xt[:, j * 512:(j + 1) * 512],
                                 start=(j == 0), stop=(j == 7))
            sbp = sb.tile([32, 128], f32, tag="sbpq")
            nc.vector.tensor_reduce(
                sbp[:], ps[:].rearrange("m (s d) -> m d s", s=4, d=128),
                axis=AX.X, op=OP.add)
            pst = tp_ps.tile([128, 32], f32, tag="tp")
            nc.tensor.matmul(pst[:], lhsT=sbp[:], rhs=id32[:], start=True, stop=True)
            nc.scalar.copy(qTall[:, h * 32:(h + 1) * 32], pst[:])

        def pool_k(h):
            xt = bigk.tile([128, 8192], f32, tag="xtk")
            nc.sync.dma_start(xt[:], ks[h].rearrange("(a b) d -> a (b d)", a=128))
            ps = pool_ps.tile([64, 512], f32, tag="pool")
            for j in range(16):
                nc.tensor.matmul(ps[:], lhsT=blkk[:], rhs=xt[:, j * 512:(j + 1) * 512],
                                 start=(j == 0), stop=(j == 15))
            sbp = sb.tile([64, 128], f32, tag="sbpk")
            nc.vector.tensor_reduce(
                sbp[:], ps[:].rearrange("m (s d) -> m d s", s=4, d=128),
                axis=AX.X, op=OP.add)
            pst = tp_ps.tile([128, 64], f32, tag="tp")
            nc.tensor.matmul(pst[:], lhsT=sbp[:], rhs=id64[:], start=True, stop=True)
            nc.scalar.copy(kTall[:, h * 64:(h + 1) * 64], pst[:])

        def mlp(xT, n0, n1, imp_dst):
            """importance MLP on xT columns [n0:n1] -> imp_dst slice (exp'd)."""
            n = n1 - n0
            ps1 = small_ps.tile([HID1, n], f32, tag="mlp")
            nc.tensor.matmul(ps1[:], lhsT=w1[:], rhs=xT[:, n0:n1], start=True, stop=True)
            h1 = sb.tile([HID1, n], f32, tag="h1")
            nc.scalar.activation(h1[:], ps1[:], AF.Relu, bias=b1[:])
            ps2 = small_ps.tile([HID2, n], f32, tag="mlp")
            nc.tensor.matmul(ps2[:], lhsT=w2[:], rhs=h1[:], start=True, stop=True)
            h2 = sb.tile([HID2, n], f32, tag="h2")
            nc.scalar.activation(h2[:], ps2[:], AF.Relu, bias=b2[:])
            ps3 = small_ps.tile([1, n], f32, tag="mlp")
            nc.tensor.matmul(ps3[:], lhsT=w3[:], rhs=h2[:], start=True, stop=True)
            nc.scalar.activation(imp_dst, ps3[:], AF.Exp, bias=nb3[:], scale=-1.0)

        def interact_grp(g):
            """4 heads hh=4g..4g+3: interaction + combine -> estage3 columns."""
            # stacked partial-interaction projections
            psq = tp_ps.tile([128, 32], f32, tag="tp")
            for cc in range(4):
                nc.tensor.matmul(psq[32 * cc:32 * cc + 32, :], lhsT=w1a[:],
                                 rhs=qTall[:, (4 * g + cc) * 32:(4 * g + cc + 1) * 32],
                                 tile_position=(0, 32 * cc), start=True, stop=True)
            qp4 = sb.tile([128, 32], f32, tag="qp4")
            nc.scalar.copy(qp4[:], psq[:])
            psk = tp_ps.tile([128, 64], f32, tag="tp")
            for cc in range(4):
                nc.tensor.matmul(psk[32 * cc:32 * cc + 32, :], lhsT=w1b[:],
                                 rhs=kTall[:, (4 * g + cc) * 64:(4 * g + cc + 1) * 64],
                                 tile_position=(0, 32 * cc), start=True, stop=True)
            kp4 = sb.tile([128, 64], f32, tag="kp4")
            nc.scalar.activation(kp4[:], psk[:], AF.Identity, bias=b1i[:])
            # grid add + relu: h[(hh,hid), (i, j)]
            hh = hpool.tile([128, 2048], f32, tag="hh")
            nc.vector.tensor_tensor(
                hh[:].rearrange("p (i j) -> p i j", i=32),
                qp4[:].unsqueeze(2).broadcast_to((128, 32, 64)),
                kp4[:].unsqueeze(1).broadcast_to((128, 32, 64)),
                op=OP.add)
            nc.scalar.activation(hh[:], hh[:], AF.Relu)
            e4 = sb.tile([4, 2048], f32, tag="e4")
            for n in range(4):
                psI = int_ps.tile([4, 512], f32, tag="int")
                nc.tensor.matmul(psI[:], lhsT=w2bd[:], rhs=hh[:, n * 512:(n + 1) * 512],
                                 start=True, stop=True)
                nc.scalar.activation(e4[:, n * 512:(n + 1) * 512], psI[:],
                                     AF.Exp, bias=nb2i[:], scale=-1.0)
            # sigma = 1/(1+e), then multiply imp factors
            nc.vector.tensor_scalar_add(e4[:], e4[:], 1.0)
            nc.vector.reciprocal(e4[:], e4[:])
            qg = sb.tile([4, 32], f32, tag="qg")
            nc.sync.dma_start(
                qg[:], qimp[0:1, g * 128:(g + 1) * 128]
                .rearrange("o (c i) -> o c i", c=4))
            kg = sb.tile([4, 64], f32, tag="kg")
            nc.sync.dma_start(
                kg[:], kimp[0:1, g * 256:(g + 1) * 256]
                .rearrange("o (c j) -> o c j", c=4))
            e3 = e4[:].rearrange("p (i j) -> p i j", i=32)
            nc.vector.tensor_tensor(e3, e3,
                                    qg[:].unsqueeze(2).broadcast_to((4, 32, 64)),
                                    op=OP.mult)
            nc.vector.tensor_tensor(e3, e3,
                                    kg[:].unsqueeze(1).broadcast_to((4, 32, 64)),
                                    op=OP.mult)
            # scatter each head row into estage3 [(32 i) p, 64 j @ hh*64]
            for cc in range(4):
                nc.sync.dma_start(
                    estage3[:, (4 * g + cc) * 64:(4 * g + cc + 1) * 64],
                    e4[cc:cc + 1, :])

        # ---- emit program ----
        for g in range(4):
            for cc in range(4):
                pool_q(4 * g + cc)
                pool_k(4 * g + cc)
            # sigmoid denominators for this grp's heads
            mlp(qTall, g * 128, (g + 1) * 128, qimp[0:1, g * 128:(g + 1) * 128])
            mlp(kTall, g * 256, (g + 1) * 256, kimp[0:1, g * 256:(g + 1) * 256])
            nc.vector.tensor_scalar_add(qimp[0:1, g * 128:(g + 1) * 128],
                                        qimp[0:1, g * 128:(g + 1) * 128], 1.0)
            nc.vector.reciprocal(qimp[0:1, g * 128:(g + 1) * 128],
                                 qimp[0:1, g * 128:(g + 1) * 128])
            nc.vector.tensor_scalar_add(kimp[0:1, g * 256:(g + 1) * 256],
                                        kimp[0:1, g * 256:(g + 1) * 256], 1.0)
            nc.vector.reciprocal(kimp[0:1, g * 256:(g + 1) * 256],
                                 kimp[0:1, g * 256:(g + 1) * 256])
            interact_grp(g)

        # free-dim transpose (hh, j) -> (j, hh)
        nc.vector.tensor_copy(
            estage4[:].rearrange("p (j hh) -> p hh j", j=64, hh=16),
            estage3[:].rearrange("p (hh j) -> p hh j", hh=16, j=64))
        # fold rows: folded[p=(a,jhalf), (r, l=(jpar,hh))]
        for rr in range(NROW):
            nc.sync.dma_start(
                folded[:, rr * 32:(rr + 1) * 32],
                estage4[4 * rr:4 * rr + 4, :]
                .rearrange("p (jh l) -> p jh l", jh=32, l=32))

        # ---- top-k threshold bisection over the 8 rows ----
        lo = persist.tile([128, NROW], f32, tag="lo")
        hi = persist.tile([128, NROW], f32, tag="hi")
        thr = persist.tile([128, NROW], f32, tag="thr")
        tmp = persist.tile([128, NROW], f32, tag="tmp")
        cntb = persist.tile([128, NROW], f32, tag="cntb")
        pred = persist.tile([128, NROW], mybir.dt.uint32, tag="pred")
        ge = persist.tile([128, NROW * 32], f32, tag="ge")
        cntp = persist.tile([128, NROW], f32, tag="cntp")
        nc.vector.memset(lo[:], 0.0)
        nc.vector.memset(hi[:], 1.0)
        nc.vector.memset(thr[:], 0.5)
        f3 = folded[:].rearrange("p (c l) -> p c l", c=NROW)
        for _ in range(NITER):
            nc.vector.tensor_tensor(
                ge[:].rearrange("p (c l) -> p c l", c=NROW), f3,
                thr[:].unsqueeze(2).broadcast_to((128, NROW, 32)), op=OP.is_ge)
            nc.vector.tensor_reduce(
                cntp[:], ge[:].rearrange("p (c l) -> p c l", c=NROW),
                axis=AX.X, op=OP.add)
            psC = small_ps.tile([128, NROW], f32, tag="mlp")
            nc.tensor.matmul(psC[:], lhsT=ones[:], rhs=cntp[:], start=True, stop=True)
            nc.scalar.copy(cntb[:], psC[:])
            nc.vector.tensor_scalar(pred[:], cntb[:], float(KSEL), None, op0=OP.is_ge)
            nc.vector.copy_predicated(lo[:], pred[:], thr[:])
            nc.vector.tensor_scalar(pred[:], cntb[:], float(KSEL), None, op0=OP.is_lt)
            nc.vector.copy_predicated(hi[:], pred[:], thr[:])
            nc.vector.tensor_add(tmp[:], lo[:], hi[:])
            nc.vector.tensor_scalar_mul(thr[:], tmp[:], 0.5)

        mask = persist.tile([128, NROW * 32], u8, tag="mask")
        nc.vector.tensor_tensor(
            mask[:].rearrange("p (c l) -> p c l", c=NROW), f3,
            lo[:].unsqueeze(2).broadcast_to((128, NROW, 32)), op=OP.is_ge)
        nc.sync.dma_start(
            y[:].rearrange("c i (jh l) -> (i jh) c l", jh=2, l=32),
            mask[:].rearrange("p (c l) -> p c l", c=NROW))

    nc.compile()
    return nc


def _constants(w_imp1, b_imp1, w_imp2, b_imp2, w_imp3, b_imp3,
               w_int1, b_int1, w_int2, b_int2):
    f = np.float32
    blkq = np.zeros((128, 32), f)
    blkq[np.arange(128), np.arange(128) // 4] = 1.0
    blkk = np.zeros((128, 64), f)
    blkk[np.arange(128), np.arange(128) // 2] = 1.0
    consts = {
        "c_blkq": blkq,
        "c_blkk": blkk,
        "c_id32": (np.eye(32, dtype=f) / 128.0).astype(f),
        "c_id64": (np.eye(64, dtype=f) / 128.0).astype(f),
        "c_ones": np.ones((128, 128), f),
        "c_w1": np.ascontiguousarray(w_imp1, f),
        "c_b1": np.ascontiguousarray(np.asarray(b_imp1, f).reshape(HID1, 1)),
        "c_w2": np.ascontiguousarray(w_imp2, f),
        "c_b2": np.ascontiguousarray(np.asarray(b_imp2, f).reshape(HID2, 1)),
        "c_w3": np.ascontiguousarray(w_imp3, f),
        "c_nb3": np.ascontiguousarray(-np.asarray(b_imp3, f).reshape(1, 1)),
        "c_w1a": np.ascontiguousarray(np.asarray(w_int1, f)[:D]),
        "c_w1b": np.ascontiguousarray(np.asarray(w_int1, f)[D:]),
        "c_b1i": np.ascontiguousarray(
            np.tile(np.asarray(b_int1, f).reshape(HID1, 1), (4, 1))),
        "c_nb2i": np.ascontiguousarray(
            np.tile(-np.asarray(b_int2, f).reshape(1, 1), (4, 1))),
    }
    w2bd = np.zeros((128, 4), f)
    for c in range(4):
        w2bd[32 * c:32 * c + 32, c] = np.asarray(w_int2, f)[:, 0]
    consts["c_w2bd"] = w2bd
    return consts


def _in_maps(q, k, w_imp1, b_imp1, w_imp2, b_imp2, w_imp3, b_imp3,
             w_int1, b_int1, w_int2, b_int2):
    q = np.asarray(q, np.float32)
    k = np.asarray(k, np.float32)
    consts = _constants(w_imp1, b_imp1, w_imp2, b_imp2, w_imp3, b_imp3,
                        w_int1, b_int1, w_int2, b_int2)
    # per-batch per-head-contiguous k; per-core half-of-q
    kT = [np.ascontiguousarray(k[b].transpose(1, 0, 2)) for b in range(B)]
    in_maps = []
    for c in range(NCORES):
        b, rg = c // 2, c % 2
        qslice = np.ascontiguousarray(
            q[b, rg * (S // 2):(rg + 1) * (S // 2)].transpose(1, 0, 2))
        m = {"qs": qslice, "ks": kT[b]}
        m.update(consts)
        in_maps.append(m)
    return in_maps


def kernel(q, k, **w):
    from concourse.bass_utils import run_bass_kernel_spmd

    in_maps = _in_maps(q, k, **w)

    if "nc" not in _nc_cache:
        _nc_cache["nc"] = _build_nc()
    res = run_bass_kernel_spmd(_nc_cache["nc"], in_maps,
                               core_ids=list(range(NCORES)))
    out = np.empty((B, H, NB, NB), np.uint8)
    for c in range(NCORES):
        b, rg = c // 2, c % 2
        out[b, rg * 8:(rg + 1) * 8] = res.results[c]["y"]
    return out > 0



# revision 21
# speedup vs baseline: 1.5075x; 1.5075x over previous
"""Trainium2 Bass kernel for ContentAdaptiveSparsity (topk_masking).

Reference semantics: combined[b,i,j,h] = q_imp[b,i,h] * k_imp[b,j,h] *
interaction[b,i,j,h] from block-mean pooled q,k (64 blocks of 128) through
tiny MLPs; raw row-major reshape [B,nb,nb,H] -> [B,16,4096]; top-1024 mask
per (b, r) row, candidate m = (i%4)*1024 + j*16 + h, out y[r, m//64, m%64].

Sharding: core c = (batch b=c//2, seq half p=c%2) reads the CONTIGUOUS
slices q[b, p*4096:...] / k[b, p*4096:...] (64 MB, zero-copy host views).
Its 32 i-blocks are exactly its 8 output rows r in [8p, 8p+8); its 32
j-blocks are half of k.  Core pairs exchange pooled-k halves (256 KB) via
pairwise AllGather through DRAM.

Device pipeline:
  - 32 per-head tile loads [128 chunks x (32 rows, 128 d)] on SP, gather
    APs straight off the raw [S,H,D] layout (512B descriptors).
  - pooling split across engines: k heads + first q heads on DVE (grouped
    free-dim reduce -> [128a,128d] partial, then one matmul lhsT=partial,
    rhs=chunk->block/128 that fuses cross-chunk sum + mean + transpose);
    remaining q heads on PE (block-diag ones matmuls accumulating
    [32,512] psum + small DVE grouped reduce + id32/128 transpose).
  - k first -> kPack -> DRAM -> AllGather(pair) -> kTall [128,(h,64)];
    per 4-head grp: importance MLPs, qk=qimp x kimp outer (off critical
    path), interaction grid add (DVE), relu (ACT), block-diag w2 matmul
    per 512-col slice pipelined with exp (ACT), +1 (ACT bias), recip
    (DVE), * qk (DVE); scatter rows to estage3, per-grp transpose copy
    (hh,j)->(j,hh) into estage4 (ACT).
  - fold: 8 DMAs estage4 -> foldedT [128=(r,a,jq), (jr,hh)] (1KB descs):
    row r occupies partitions 16r..16r+16; its 4096 candidates are
    (a,jq | jr,hh) with j = jq*16+jr.
  - top-k: two-phase exact fp32 bisection from (0,1], compile-time
    power-of-2 steps (thr sums EXACT in fp32).  Per iter: ONE fused DVE
    tensor_tensor_reduce (compare >= thr, per-partition count), blk16
    block-diag matmul summing each row's 16 partitions (result replicated
    across the group), tensor_scalar + scalar_tensor_tensor threshold
    update.  After 18 iters bracket width 2^-18; rebase foldedT2 =
    (foldedT - L)*2^18 in ONE fused op (Sterbenz-exact near the bracket),
    12 more iters -> resolution 2^-30, far under the min candidate gap
    (~1.5e-8).  Mask = (foldedT2 >= L2) via per-partition scalar compare.
"""

import numpy as np

B, S, H, D = 4, 8192, 16, 128
SH = S // 2       # per-core seq slice
NB = 64           # blocks per sequence
NBH = 32          # blocks per core half
NROW = 8          # topk rows (r) per core
NCORES = 8
KSEL = 1024
HID1, HID2 = 32, 16
NIT1, NIT2 = 18, 12
N_Q_DVE = 4       # q heads pooled on the DVE path (rest on PE)

_nc_cache = {}


def _build_nc():
    from contextlib import ExitStack

    from concourse import bacc
    import concourse.mybir as mybir
    from concourse.tile import TileContext

    f32 = mybir.dt.float32
    u8 = mybir.dt.uint8
    AF = mybir.ActivationFunctionType
    OP = mybir.AluOpType
    AX = mybir.AxisListType

    nc = bacc.Bacc("TRN2", target_bir_lowering=False, debug=False,
                   num_devices=NCORES)

    qs = nc.dram_tensor("qs", [SH, H, D], f32, kind="ExternalInput")
    ks = nc.dram_tensor("ks", [SH, H, D], f32, kind="ExternalInput")
    c_blkS = nc.dram_tensor("c_blkS", [128, NBH], f32, kind="ExternalInput")
    c_blkq = nc.dram_tensor("c_blkq", [128, 32], f32, kind="ExternalInput")
    c_id32 = nc.dram_tensor("c_id32", [32, 32], f32, kind="ExternalInput")
    c_blk16 = nc.dram_tensor("c_blk16", [128, 128], f32, kind="ExternalInput")
    c_w1 = nc.dram_tensor("c_w1", [D, HID1], f32, kind="ExternalInput")
    c_b1 = nc.dram_tensor("c_b1", [HID1, 1], f32, kind="ExternalInput")
    c_w2 = nc.dram_tensor("c_w2", [HID1, HID2], f32, kind="ExternalInput")
    c_b2 = nc.dram_tensor("c_b2", [HID2, 1], f32, kind="ExternalInput")
    c_w3 = nc.dram_tensor("c_w3", [HID2, 1], f32, kind="ExternalInput")
    c_nb3 = nc.dram_tensor("c_nb3", [1, 1], f32, kind="ExternalInput")
    c_w1a = nc.dram_tensor("c_w1a", [D, HID1], f32, kind="ExternalInput")
    c_w1b = nc.dram_tensor("c_w1b", [D, HID1], f32, kind="ExternalInput")
    c_b1i = nc.dram_tensor("c_b1i", [128, 1], f32, kind="ExternalInput")
    c_w2bd = nc.dram_tensor("c_w2bd", [128, 4], f32, kind="ExternalInput")
    c_nb2i = nc.dram_tensor("c_nb2i", [4, 1], f32, kind="ExternalInput")
    c_one4 = nc.dram_tensor("c_one4", [4, 1], f32, kind="ExternalInput")

    y = nc.dram_tensor("y", [NROW, NB, NB], u8, kind="ExternalOutput")

    with TileContext(nc) as tc, ExitStack() as ctx:
        const = ctx.enter_context(tc.tile_pool(name="const", bufs=1))
        bigk = ctx.enter_context(tc.tile_pool(name="bigk", bufs=3))
        bigq = ctx.enter_context(tc.tile_pool(name="bigq", bufs=4))
        part_pool = ctx.enter_context(tc.tile_pool(name="part", bufs=3))
        hpool = ctx.enter_context(tc.tile_pool(name="hpool", bufs=2))
        sb = ctx.enter_context(tc.tile_pool(name="sb", bufs=2))
        persist = ctx.enter_context(tc.tile_pool(name="persist", bufs=1))
        dram = ctx.enter_context(tc.tile_pool(name="dram", bufs=1, space="DRAM"))
        pool_ps = ctx.enter_context(tc.tile_pool(name="pool_ps", bufs=1, space="PSUM"))
        pool2_ps = ctx.enter_context(tc.tile_pool(name="pool2_ps", bufs=2, space="PSUM"))
        tp_ps = ctx.enter_context(tc.tile_pool(name="tp_ps", bufs=2, space="PSUM"))
        small_ps = ctx.enter_context(tc.tile_pool(name="small_ps", bufs=1, space="PSUM"))
        int_ps = ctx.enter_context(tc.tile_pool(name="int_ps", bufs=2, space="PSUM"))

        def cload(dramt, shape, tag):
            t = const.tile(shape, f32, tag=tag)
            nc.gpsimd.dma_start(t[:], dramt[:])
            return t

        blkS = cload(c_blkS, [128, NBH], "blkS")
        blkq = cload(c_blkq, [128, 32], "blkq")
        id32 = cload(c_id32, [32, 32], "id32")
        blk16 = cload(c_blk16, [128, 128], "blk16")
        w1 = cload(c_w1, [D, HID1], "w1")
        b1 = cload(c_b1, [HID1, 1], "b1")
        w2 = cload(c_w2, [HID1, HID2], "w2")
        b2 = cload(c_b2, [HID2, 1], "b2")
        w3 = cload(c_w3, [HID2, 1], "w3")
        nb3 = cload(c_nb3, [1, 1], "nb3")
        w1a = cload(c_w1a, [D, HID1], "w1a")
        w1b = cload(c_w1b, [D, HID1], "w1b")
        b1i = cload(c_b1i, [128, 1], "b1i")
        w2bd = cload(c_w2bd, [128, 4], "w2bd")
        nb2i = cload(c_nb2i, [4, 1], "nb2i")
        one4 = cload(c_one4, [4, 1], "one4")

        qTall = persist.tile([128, H * NBH], f32, tag="qTall")    # (h, i32)
        kPack = persist.tile([128, H * NBH], f32, tag="kPack")    # (h, j32) mine
        kTall = persist.tile([128, H * NB], f32, tag="kTall")     # (h, j64)
        ktmp = persist.tile([128, H * NB], f32, tag="ktmp")       # (half, h, j32)
        qimp = persist.tile([1, H * NBH], f32, tag="qimp")
        kimp = persist.tile([1, H * NB], f32, tag="kimp")
        estage3 = persist.tile([NBH, H * NB], f32, tag="estage3")  # [i, (hh,j)]
        estage4 = persist.tile([NBH, H * NB], f32, tag="estage4")  # [i, (j,hh)]
        foldedT = persist.tile([128, 256], f32, tag="foldedT")
        foldedT2 = persist.tile([128, 256], f32, tag="foldedT2")

        cc_in = dram.tile([128, H * NBH], f32)
        cc_out = dram.tile([2 * 128, H * NBH], f32)

        def pool_dve(src, h, big, dst, dtag):
            """DVE-path block-mean pool of head h -> dst cols [h*32,(h+1)*32)."""
            xt = big.tile([128, 4096], f32, tag=dtag)
            nc.sync.dma_start(
                xt[:].rearrange("a (b d) -> a b d", b=32),
                src[:, h, :].rearrange("(a b) d -> a b d", a=128))
            part = part_pool.tile([128, 128], f32, tag="part")
            nc.vector.tensor_reduce(
                part[:], xt[:].rearrange("a (b d) -> a d b", b=32),
                axis=AX.X, op=OP.add)
            ps = pool_ps.tile([128, NBH], f32, tag="pool")
            nc.tensor.matmul(ps[:], lhsT=part[:], rhs=blkS[:],
                             start=True, stop=True)
            nc.scalar.copy(dst[:, h * NBH:(h + 1) * NBH], ps[:])

        def pool_pe(src, h, big, dst, dtag):
            """PE-path block-mean pool of head h -> dst cols [h*32,(h+1)*32)."""
            xt = big.tile([128, 4096], f32, tag=dtag)
            nc.sync.dma_start(
                xt[:].rearrange("a (b d) -> a b d", b=32),
                src[:, h, :].rearrange("(a b) d -> a b d", a=128))
            ps = pool2_ps.tile([32, 512], f32, tag="pool2")
            for j in range(8):
                nc.tensor.matmul(ps[:], lhsT=blkq[:], rhs=xt[:, j * 512:(j + 1) * 512],
                                 start=(j == 0), stop=(j == 7))
            sbp = sb.tile([32, 128], f32, tag="sbp")
            nc.vector.tensor_reduce(
                sbp[:], ps[:].rearrange("m (s d) -> m d s", s=4, d=128),
                axis=AX.X, op=OP.add)
            pst = tp_ps.tile([128, NBH], f32, tag="tp")
            nc.tensor.matmul(pst[:], lhsT=sbp[:], rhs=id32[:], start=True, stop=True)
            nc.scalar.copy(dst[:, h * NBH:(h + 1) * NBH], pst[:])

        def mlp(xT, n0, n1, imp_dst):
            """importance MLP on xT cols [n0:n1] -> imp_dst (exp(-logit))."""
            n = n1 - n0
            ps1 = small_ps.tile([HID1, n], f32, tag="mlp")
            nc.tensor.matmul(ps1[:], lhsT=w1[:], rhs=xT[:, n0:n1], start=True, stop=True)
            h1 = sb.tile([HID1, n], f32, tag="h1")
            nc.scalar.activation(h1[:], ps1[:], AF.Relu, bias=b1[:])
            ps2 = small_ps.tile([HID2, n], f32, tag="mlp")
            nc.tensor.matmul(ps2[:], lhsT=w2[:], rhs=h1[:], start=True, stop=True)
            h2 = sb.tile([HID2, n], f32, tag="h2")
            nc.scalar.activation(h2[:], ps2[:], AF.Relu, bias=b2[:])
            ps3 = small_ps.tile([1, n], f32, tag="mlp")
            nc.tensor.matmul(ps3[:], lhsT=w3[:], rhs=h2[:], start=True, stop=True)
            nc.scalar.activation(imp_dst, ps3[:], AF.Exp, bias=nb3[:], scale=-1.0)

        def sigma_inplace(ap):
            nc.vector.tensor_scalar_add(ap, ap, 1.0)
            nc.vector.reciprocal(ap, ap)

        def interact_grp(g):
            """heads hh=4g..4g+3: interaction + combine -> estage3/estage4."""
            # qk = qimp (x) kimp outer product, off the critical path
            qg = sb.tile([4, NBH], f32, tag="qg")
            nc.gpsimd.dma_start(
                qg[:], qimp[0:1, g * 128:(g + 1) * 128]
                .rearrange("o (c i) -> o c i", c=4))
            kg = sb.tile([4, NB], f32, tag="kg")
            nc.gpsimd.dma_start(
                kg[:], kimp[0:1, g * 256:(g + 1) * 256]
                .rearrange("o (c j) -> o c j", c=4))
            qk = sb.tile([4, NBH * NB], f32, tag="qk")
            nc.vector.tensor_tensor(
                qk[:].rearrange("p (i j) -> p i j", i=NBH),
                qg[:].unsqueeze(2).broadcast_to((4, NBH, NB)),
                kg[:].unsqueeze(1).broadcast_to((4, NBH, NB)),
                op=OP.mult)

            psq = tp_ps.tile([128, NBH], f32, tag="tp")
            for cc in range(4):
                nc.tensor.matmul(psq[32 * cc:32 * cc + 32, :], lhsT=w1a[:],
                                 rhs=qTall[:, (4 * g + cc) * NBH:(4 * g + cc + 1) * NBH],
                                 tile_position=(0, 32 * cc), start=True, stop=True)
            qp4 = sb.tile([128, NBH], f32, tag="qp4")
            nc.scalar.copy(qp4[:], psq[:])
            psk = tp_ps.tile([128, NB], f32, tag="tp")
            for cc in range(4):
                nc.tensor.matmul(psk[32 * cc:32 * cc + 32, :], lhsT=w1b[:],
                                 rhs=kTall[:, (4 * g + cc) * NB:(4 * g + cc + 1) * NB],
                                 tile_position=(0, 32 * cc), start=True, stop=True)
            kp4 = sb.tile([128, NB], f32, tag="kp4")
            nc.scalar.activation(kp4[:], psk[:], AF.Identity, bias=b1i[:])
            hh = hpool.tile([128, NBH * NB], f32, tag="hh")
            nc.vector.tensor_tensor(
                hh[:].rearrange("p (i j) -> p i j", i=NBH),
                qp4[:].unsqueeze(2).broadcast_to((128, NBH, NB)),
                kp4[:].unsqueeze(1).broadcast_to((128, NBH, NB)),
                op=OP.add)
            nc.scalar.activation(hh[:], hh[:], AF.Relu)
            e4 = sb.tile([4, NBH * NB], f32, tag="e4")
            for n in range(4):
                sl = slice(n * 512, (n + 1) * 512)
                psI = int_ps.tile([4, 512], f32, tag="int")
                nc.tensor.matmul(psI[:], lhsT=w2bd[:], rhs=hh[:, sl],
                                 start=True, stop=True)
                nc.scalar.activation(e4[:, sl], psI[:], AF.Exp,
                                     bias=nb2i[:], scale=-1.0)
                nc.scalar.activation(e4[:, sl], e4[:, sl], AF.Identity,
                                     bias=one4[:])
                nc.vector.reciprocal(e4[:, sl], e4[:, sl])
                nc.vector.tensor_tensor(e4[:, sl], e4[:, sl], qk[:, sl],
                                        op=OP.mult)
            # scatter head rows into estage3 [(32 i) p, 64 j @ hh*64]
            for cc in range(4):
                nc.scalar.dma_start(
                    estage3[:, (4 * g + cc) * NB:(4 * g + cc + 1) * NB],
                    e4[cc:cc + 1, :])
            # incremental (hh,j)->(j,hh) transpose for this grp's columns
            nc.scalar.copy(
                estage4[:].rearrange("p (j hh) -> p hh j", j=NB, hh=16)
                [:, 4 * g:4 * g + 4, :],
                estage3[:, 4 * g * NB:(4 * g + 4) * NB]
                .rearrange("p (hh j) -> p hh j", hh=4, j=NB))

        # ---- emit program ----
        for h in range(H):
            pool_dve(ks, h, bigk, kPack, "k")
        nc.gpsimd.dma_start(cc_in[:], kPack[:])
        nc.gpsimd.collective_compute(
            "AllGather", mybir.AluOpType.bypass,
            replica_groups=[[0, 1], [2, 3], [4, 5], [6, 7]],
            ins=[cc_in.opt()], outs=[cc_out.opt()])
        nc.gpsimd.dma_start(ktmp[:, 0:H * NBH], cc_out[0:128])
        nc.gpsimd.dma_start(ktmp[:, H * NBH:], cc_out[128:256])
        # interleave on ACT: kTall[(h, half, j)] <- ktmp[(half, h, j)]
        nc.scalar.copy(
            kTall[:].rearrange("p (h c j) -> p c h j", h=H, c=2, j=NBH),
            ktmp[:].rearrange("p (c h j) -> p c h j", c=2, h=H, j=NBH))

        nq_dve = 0
        for g in range(4):
            for cc in range(4):
                h = 4 * g + cc
                if nq_dve < N_Q_DVE:
                    pool_dve(qs, h, bigq, qTall, "q")
                    nq_dve += 1
                else:
                    pool_pe(qs, h, bigq, qTall, "q")
            mlp(qTall, g * 128, (g + 1) * 128, qimp[0:1, g * 128:(g + 1) * 128])
            mlp(kTall, g * 256, (g + 1) * 256, kimp[0:1, g * 256:(g + 1) * 256])
            sigma_inplace(qimp[0:1, g * 128:(g + 1) * 128])
            sigma_inplace(kimp[0:1, g * 256:(g + 1) * 256])
            interact_grp(g)

        # fold into foldedT [p=(r,a,jq), (jr,hh)], j = jq*16 + jr
        for rr in range(NROW):
            eng = nc.sync if rr % 2 == 0 else nc.scalar
            eng.dma_start(
                foldedT[16 * rr:16 * rr + 16, :]
                .rearrange("p (jr hh) -> p jr hh", jr=16, hh=16),
                estage4[4 * rr:4 * rr + 4, :]
                .rearrange("p (jq jr hh) -> p jq jr hh", jq=4, jr=16, hh=16))

        # ---- top-k: two-phase exact fp32 bisection (transposed layout) ----
        thr = persist.tile([128, 1], f32, tag="thr")
        pr = persist.tile([128, 1], f32, tag="pr")
        lo = persist.tile([128, 1], f32, tag="lo")
        ge = persist.tile([128, 256], f32, tag="ge")
        cntp = persist.tile([128, 1], f32, tag="cntp")

        def bisect(vals, nit):
            nc.vector.memset(thr[:], 0.5)
            s = 0.25
            for _ in range(nit):
                psC = small_ps.tile([128, 1], f32, tag="mlp")
                nc.vector.tensor_tensor(
                    ge[:], vals[:], thr[:].broadcast_to((128, 256)),
                    op=OP.is_ge)
                nc.vector.tensor_reduce(cntp[:], ge[:], axis=AX.X, op=OP.add)
                nc.tensor.matmul(psC[:], lhsT=blk16[:], rhs=cntp[:],
                                 start=True, stop=True)
                # pr = (cnt >= K) * 2s ; thr += pr - s
                nc.vector.tensor_scalar(pr[:], psC[:], float(KSEL), 2.0 * s,
                                        op0=OP.is_ge, op1=OP.mult)
                nc.vector.scalar_tensor_tensor(thr[:], pr[:], -s, thr[:],
                                               op0=OP.add, op1=OP.add)
                s *= 0.5
            return s

        s1 = bisect(foldedT, NIT1)
        nc.vector.tensor_scalar_add(lo[:], thr[:], -2.0 * s1)
        nc.vector.tensor_scalar(foldedT2[:], foldedT[:], lo[:],
                                float(2 ** NIT1), op0=OP.subtract, op1=OP.mult)
        s2 = bisect(foldedT2, NIT2)
        nc.vector.tensor_scalar_add(lo[:], thr[:], -2.0 * s2)

        mask = persist.tile([128, 256], u8, tag="mask")
        nc.vector.tensor_scalar(mask[:], foldedT2[:], lo[:], None, op0=OP.is_ge)
        nc.sync.dma_start(
            y[:].rearrange("r (a jq jr4) (jrm hh) -> (r a jq) jr4 jrm hh",
                           a=4, jq=4, jr4=4, jrm=4, hh=16),
            mask[:].rearrange("p (jr4 jrm hh) -> p jr4 jrm hh",
                              jr4=4, jrm=4, hh=16))

    nc.compile()
    return nc


def _constants(w_imp1, b_imp1, w_imp2, b_imp2, w_imp3, b_imp3,
               w_int1, b_int1, w_int2, b_int2):
    f = np.float32
    blkS = np.zeros((128, NBH), f)
    blkS[np.arange(128), np.arange(128) // 4] = 1.0 / 128.0
    blkq = np.zeros((128, 32), f)
    blkq[np.arange(128), np.arange(128) // 4] = 1.0
    blk16 = np.zeros((128, 128), f)
    blk16[np.arange(128)[:, None] // 16 == np.arange(128)[None, :] // 16] = 1.0
    consts = {
        "c_blkS": blkS,
        "c_blkq": blkq,
        "c_id32": (np.eye(32, dtype=f) / 128.0).astype(f),
        "c_blk16": blk16,
        "c_w1": np.ascontiguousarray(w_imp1, f),
        "c_b1": np.ascontiguousarray(np.asarray(b_imp1, f).reshape(HID1, 1)),
        "c_w2": np.ascontiguousarray(w_imp2, f),
        "c_b2": np.ascontiguousarray(np.asarray(b_imp2, f).reshape(HID2, 1)),
        "c_w3": np.ascontiguousarray(w_imp3, f),
        "c_nb3": np.ascontiguousarray(-np.asarray(b_imp3, f).reshape(1, 1)),
        "c_w1a": np.ascontiguousarray(np.asarray(w_int1, f)[:D]),
        "c_w1b": np.ascontiguousarray(np.asarray(w_int1, f)[D:]),
        "c_b1i": np.ascontiguousarray(
            np.tile(np.asarray(b_int1, f).reshape(HID1, 1), (4, 1))),
        "c_nb2i": np.ascontiguousarray(
            np.tile(-np.asarray(b_int2, f).reshape(1, 1), (4, 1))),
        "c_one4": np.ones((4, 1), f),
    }
    w2bd = np.zeros((128, 4), f)
    for c in range(4):
        w2bd[32 * c:32 * c + 32, c] = np.asarray(w_int2, f)[:, 0]
    consts["c_w2bd"] = w2bd
    return consts


def _in_maps(q, k, w_imp1, b_imp1, w_imp2, b_imp2, w_imp3, b_imp3,
             w_int1, b_int1, w_int2, b_int2):
    q = np.asarray(q, np.float32)
    k = np.asarray(k, np.float32)
    consts = _constants(w_imp1, b_imp1, w_imp2, b_imp2, w_imp3, b_imp3,
                        w_int1, b_int1, w_int2, b_int2)
    in_maps = []
    for c in range(NCORES):
        b, p = c // 2, c % 2
        m = {"qs": q[b, p * SH:(p + 1) * SH],
             "ks": k[b, p * SH:(p + 1) * SH]}
        m.update(consts)
        in_maps.append(m)
    return in_maps


def kernel(q, k, **w):
    from concourse.bass_utils import run_bass_kernel_spmd

    in_maps = _in_maps(q, k, **w)

    if "nc" not in _nc_cache:
        _nc_cache["nc"] = _build_nc()
    res = run_bass_kernel_spmd(_nc_cache["nc"], in_maps,
                               core_ids=list(range(NCORES)))
    out = np.empty((B, H, NB, NB), np.uint8)
    for c in range(NCORES):
        b, p = c // 2, c % 2
        out[b, p * 8:(p + 1) * 8] = res.results[c]["y"]
    return out > 0


# revision 24
# speedup vs baseline: 1.5662x; 1.0389x over previous
"""Trainium2 Bass kernel for ContentAdaptiveSparsity (topk_masking).

Reference semantics: combined[b,i,j,h] = q_imp[b,i,h] * k_imp[b,j,h] *
interaction[b,i,j,h] from block-mean pooled q,k (64 blocks of 128) through
tiny MLPs; raw row-major reshape [B,nb,nb,H] -> [B,16,4096]; top-1024 mask
per (b, r) row, candidate m = (i%4)*1024 + j*16 + h, out y[r, m//64, m%64].

Sharding: core c = (batch b=c//2, seq half p=c%2) reads the CONTIGUOUS
slices q[b, p*4096:...] / k[b, p*4096:...] (64 MB, zero-copy host views).
Its 32 i-blocks are exactly its 8 output rows r in [8p, 8p+8); its 32
j-blocks are half of k.  Core pairs exchange pooled-k halves (256 KB) via
pairwise AllGather through DRAM.

Device pipeline:
  - 32 per-head tile loads [128 chunks x (32 rows, 128 d)] on SP, gather
    APs straight off the raw [S,H,D] layout (512B descriptors).
  - pooling split across engines: k heads + first q heads on DVE (grouped
    free-dim reduce -> [128a,128d] partial, then one matmul lhsT=partial,
    rhs=chunk->block/128 that fuses cross-chunk sum + mean + transpose);
    remaining q heads on PE (block-diag ones matmuls accumulating
    [32,512] psum + small DVE grouped reduce + id32/128 transpose).
  - k first -> kPack -> DRAM -> AllGather(pair) -> kTall [128,(h,64)];
    per 4-head grp: importance MLPs, qk=qimp x kimp outer (off critical
    path), interaction grid add (DVE), relu (ACT), block-diag w2 matmul
    per 512-col slice pipelined with exp (ACT), +1 (ACT bias), recip
    (DVE), * qk (DVE); scatter rows to estage3, per-grp transpose copy
    (hh,j)->(j,hh) into estage4 (ACT).
  - fold: 8 DMAs estage4 -> foldedT [128=(r,a,jq), (jr,hh)] (1KB descs):
    row r occupies partitions 16r..16r+16; its 4096 candidates are
    (a,jq | jr,hh) with j = jq*16+jr.
  - top-k: two-phase exact fp32 bisection from (0,1], compile-time
    power-of-2 steps (thr sums EXACT in fp32).  Per iter: ONE fused DVE
    tensor_tensor_reduce (compare >= thr, per-partition count), blk16
    block-diag matmul summing each row's 16 partitions (result replicated
    across the group), tensor_scalar + scalar_tensor_tensor threshold
    update.  After 18 iters bracket width 2^-18; rebase foldedT2 =
    (foldedT - L)*2^18 in ONE fused op (Sterbenz-exact near the bracket),
    12 more iters -> resolution 2^-30, far under the min candidate gap
    (~1.5e-8).  Mask = (foldedT2 >= L2) via per-partition scalar compare.
"""

import numpy as np

B, S, H, D = 4, 8192, 16, 128
SH = S // 2       # per-core seq slice
NB = 64           # blocks per sequence
NBH = 32          # blocks per core half
NROW = 8          # topk rows (r) per core
NCORES = 8
KSEL = 1024
HID1, HID2 = 32, 16
NIT1, NIT2 = 17, 11
N_Q_DVE = 4       # q heads pooled on the DVE path (rest on PE)

_nc_cache = {}


def _build_nc():
    from contextlib import ExitStack

    from concourse import bacc
    import concourse.mybir as mybir
    from concourse.tile import TileContext

    f32 = mybir.dt.float32
    u8 = mybir.dt.uint8
    AF = mybir.ActivationFunctionType
    OP = mybir.AluOpType
    AX = mybir.AxisListType

    nc = bacc.Bacc("TRN2", target_bir_lowering=False, debug=False,
                   num_devices=NCORES)

    qs = nc.dram_tensor("qs", [SH, H, D], f32, kind="ExternalInput")
    ks = nc.dram_tensor("ks", [SH, H, D], f32, kind="ExternalInput")
    c_blkS = nc.dram_tensor("c_blkS", [128, NBH], f32, kind="ExternalInput")
    c_blkq = nc.dram_tensor("c_blkq", [128, 32], f32, kind="ExternalInput")
    c_id32 = nc.dram_tensor("c_id32", [32, 32], f32, kind="ExternalInput")
    c_blk16 = nc.dram_tensor("c_blk16", [128, 128], f32, kind="ExternalInput")
    c_w1 = nc.dram_tensor("c_w1", [D, HID1], f32, kind="ExternalInput")
    c_b1 = nc.dram_tensor("c_b1", [HID1, 1], f32, kind="ExternalInput")
    c_w2 = nc.dram_tensor("c_w2", [HID1, HID2], f32, kind="ExternalInput")
    c_b2 = nc.dram_tensor("c_b2", [HID2, 1], f32, kind="ExternalInput")
    c_w3 = nc.dram_tensor("c_w3", [HID2, 1], f32, kind="ExternalInput")
    c_nb3 = nc.dram_tensor("c_nb3", [1, 1], f32, kind="ExternalInput")
    c_w1a = nc.dram_tensor("c_w1a", [D, HID1], f32, kind="ExternalInput")
    c_w1b = nc.dram_tensor("c_w1b", [D, HID1], f32, kind="ExternalInput")
    c_b1i = nc.dram_tensor("c_b1i", [128, 1], f32, kind="ExternalInput")
    c_w2bd = nc.dram_tensor("c_w2bd", [128, 4], f32, kind="ExternalInput")
    c_nb2i = nc.dram_tensor("c_nb2i", [4, 1], f32, kind="ExternalInput")
    c_one4 = nc.dram_tensor("c_one4", [4, 1], f32, kind="ExternalInput")

    y = nc.dram_tensor("y", [NROW, NB, NB], u8, kind="ExternalOutput")

    with TileContext(nc) as tc, ExitStack() as ctx:
        const = ctx.enter_context(tc.tile_pool(name="const", bufs=1))
        bigk = ctx.enter_context(tc.tile_pool(name="bigk", bufs=3))
        bigq = ctx.enter_context(tc.tile_pool(name="bigq", bufs=4))
        part_pool = ctx.enter_context(tc.tile_pool(name="part", bufs=3))
        hpool = ctx.enter_context(tc.tile_pool(name="hpool", bufs=2))
        sb = ctx.enter_context(tc.tile_pool(name="sb", bufs=2))
        persist = ctx.enter_context(tc.tile_pool(name="persist", bufs=1))
        dram = ctx.enter_context(tc.tile_pool(name="dram", bufs=1, space="DRAM"))
        pool_ps = ctx.enter_context(tc.tile_pool(name="pool_ps", bufs=1, space="PSUM"))
        pool2_ps = ctx.enter_context(tc.tile_pool(name="pool2_ps", bufs=2, space="PSUM"))
        tp_ps = ctx.enter_context(tc.tile_pool(name="tp_ps", bufs=2, space="PSUM"))
        small_ps = ctx.enter_context(tc.tile_pool(name="small_ps", bufs=1, space="PSUM"))
        int_ps = ctx.enter_context(tc.tile_pool(name="int_ps", bufs=2, space="PSUM"))

        def cload(dramt, shape, tag):
            t = const.tile(shape, f32, tag=tag)
            nc.gpsimd.dma_start(t[:], dramt[:])
            return t

        blkS = cload(c_blkS, [128, NBH], "blkS")
        blkq = cload(c_blkq, [128, 32], "blkq")
        id32 = cload(c_id32, [32, 32], "id32")
        blk16 = cload(c_blk16, [128, 128], "blk16")
        w1 = cload(c_w1, [D, HID1], "w1")
        b1 = cload(c_b1, [HID1, 1], "b1")
        w2 = cload(c_w2, [HID1, HID2], "w2")
        b2 = cload(c_b2, [HID2, 1], "b2")
        w3 = cload(c_w3, [HID2, 1], "w3")
        nb3 = cload(c_nb3, [1, 1], "nb3")
        w1a = cload(c_w1a, [D, HID1], "w1a")
        w1b = cload(c_w1b, [D, HID1], "w1b")
        b1i = cload(c_b1i, [128, 1], "b1i")
        w2bd = cload(c_w2bd, [128, 4], "w2bd")
        nb2i = cload(c_nb2i, [4, 1], "nb2i")
        one4 = cload(c_one4, [4, 1], "one4")

        qTall = persist.tile([128, H * NBH], f32, tag="qTall")    # (h, i32)
        kPack = persist.tile([128, H * NBH], f32, tag="kPack")    # (h, j32) mine
        kTall = persist.tile([128, H * NB], f32, tag="kTall")     # (h, j64)
        ktmp = persist.tile([128, H * NB], f32, tag="ktmp")       # (half, h, j32)
        qimp = persist.tile([1, H * NBH], f32, tag="qimp")
        kimp = persist.tile([1, H * NB], f32, tag="kimp")
        estage3 = persist.tile([NBH, H * NB], f32, tag="estage3")  # [i, (hh,j)]
        estage4 = persist.tile([NBH, H * NB], f32, tag="estage4")  # [i, (j,hh)]
        foldedT = persist.tile([128, 256], f32, tag="foldedT")
        foldedT2 = persist.tile([128, 256], f32, tag="foldedT2")

        cc_in = dram.tile([128, H * NBH], f32)
        cc_out = dram.tile([2 * 128, H * NBH], f32)

        def pool_dve(src, h, big, dst, dtag):
            """DVE-path block-mean pool of head h -> dst cols [h*32,(h+1)*32)."""
            xt = big.tile([128, 4096], f32, tag=dtag)
            nc.sync.dma_start(
                xt[:].rearrange("a (b d) -> a b d", b=32),
                src[:, h, :].rearrange("(a b) d -> a b d", a=128))
            part = part_pool.tile([128, 128], f32, tag="part")
            nc.vector.tensor_reduce(
                part[:], xt[:].rearrange("a (b d) -> a d b", b=32),
                axis=AX.X, op=OP.add)
            ps = pool_ps.tile([128, NBH], f32, tag="pool")
            nc.tensor.matmul(ps[:], lhsT=part[:], rhs=blkS[:],
                             start=True, stop=True)
            nc.scalar.copy(dst[:, h * NBH:(h + 1) * NBH], ps[:])

        def pool_pe(src, h, big, dst, dtag):
            """PE-path block-mean pool of head h -> dst cols [h*32,(h+1)*32)."""
            xt = big.tile([128, 4096], f32, tag=dtag)
            nc.sync.dma_start(
                xt[:].rearrange("a (b d) -> a b d", b=32),
                src[:, h, :].rearrange("(a b) d -> a b d", a=128))
            ps = pool2_ps.tile([32, 512], f32, tag="pool2")
            for j in range(8):
                nc.tensor.matmul(ps[:], lhsT=blkq[:], rhs=xt[:, j * 512:(j + 1) * 512],
                                 start=(j == 0), stop=(j == 7))
            sbp = sb.tile([32, 128], f32, tag="sbp")
            nc.vector.tensor_reduce(
                sbp[:], ps[:].rearrange("m (s d) -> m d s", s=4, d=128),
                axis=AX.X, op=OP.add)
            pst = tp_ps.tile([128, NBH], f32, tag="tp")
            nc.tensor.matmul(pst[:], lhsT=sbp[:], rhs=id32[:], start=True, stop=True)
            nc.scalar.copy(dst[:, h * NBH:(h + 1) * NBH], pst[:])

        def mlp(xT, n0, n1, imp_dst):
            """importance MLP on xT cols [n0:n1] -> imp_dst (exp(-logit))."""
            n = n1 - n0
            ps1 = small_ps.tile([HID1, n], f32, tag="mlp")
            nc.tensor.matmul(ps1[:], lhsT=w1[:], rhs=xT[:, n0:n1], start=True, stop=True)
            h1 = sb.tile([HID1, n], f32, tag="h1")
            nc.scalar.activation(h1[:], ps1[:], AF.Relu, bias=b1[:])
            ps2 = small_ps.tile([HID2, n], f32, tag="mlp")
            nc.tensor.matmul(ps2[:], lhsT=w2[:], rhs=h1[:], start=True, stop=True)
            h2 = sb.tile([HID2, n], f32, tag="h2")
            nc.scalar.activation(h2[:], ps2[:], AF.Relu, bias=b2[:])
            ps3 = small_ps.tile([1, n], f32, tag="mlp")
            nc.tensor.matmul(ps3[:], lhsT=w3[:], rhs=h2[:], start=True, stop=True)
            nc.scalar.activation(imp_dst, ps3[:], AF.Exp, bias=nb3[:], scale=-1.0)

        def sigma_inplace(ap):
            nc.vector.tensor_scalar_add(ap, ap, 1.0)
            nc.vector.reciprocal(ap, ap)

        def interact_grp(g):
            """heads hh=4g..4g+3: interaction + combine -> estage3/estage4."""
            # qk = qimp (x) kimp outer product, off the critical path
            qg = sb.tile([4, NBH], f32, tag="qg")
            nc.gpsimd.dma_start(
                qg[:], qimp[0:1, g * 128:(g + 1) * 128]
                .rearrange("o (c i) -> o c i", c=4))
            kg = sb.tile([4, NB], f32, tag="kg")
            nc.gpsimd.dma_start(
                kg[:], kimp[0:1, g * 256:(g + 1) * 256]
                .rearrange("o (c j) -> o c j", c=4))
            qk = sb.tile([4, NBH * NB], f32, tag="qk")
            nc.vector.tensor_tensor(
                qk[:].rearrange("p (i j) -> p i j", i=NBH),
                qg[:].unsqueeze(2).broadcast_to((4, NBH, NB)),
                kg[:].unsqueeze(1).broadcast_to((4, NBH, NB)),
                op=OP.mult)

            psq = tp_ps.tile([128, NBH], f32, tag="tp")
            for cc in range(4):
                nc.tensor.matmul(psq[32 * cc:32 * cc + 32, :], lhsT=w1a[:],
                                 rhs=qTall[:, (4 * g + cc) * NBH:(4 * g + cc + 1) * NBH],
                                 tile_position=(0, 32 * cc), start=True, stop=True)
            qp4 = sb.tile([128, NBH], f32, tag="qp4")
            nc.scalar.copy(qp4[:], psq[:])
            psk = tp_ps.tile([128, NB], f32, tag="tp")
            for cc in range(4):
                nc.tensor.matmul(psk[32 * cc:32 * cc + 32, :], lhsT=w1b[:],
                                 rhs=kTall[:, (4 * g + cc) * NB:(4 * g + cc + 1) * NB],
                                 tile_position=(0, 32 * cc), start=True, stop=True)
            kp4 = sb.tile([128, NB], f32, tag="kp4")
            nc.scalar.activation(kp4[:], psk[:], AF.Identity, bias=b1i[:])
            hh = hpool.tile([128, NBH * NB], f32, tag="hh")
            nc.vector.tensor_tensor(
                hh[:].rearrange("p (i j) -> p i j", i=NBH),
                qp4[:].unsqueeze(2).broadcast_to((128, NBH, NB)),
                kp4[:].unsqueeze(1).broadcast_to((128, NBH, NB)),
                op=OP.add)
            nc.scalar.activation(hh[:], hh[:], AF.Relu)
            e4 = sb.tile([4, NBH * NB], f32, tag="e4")
            for n in range(4):
                sl = slice(n * 512, (n + 1) * 512)
                psI = int_ps.tile([4, 512], f32, tag="int")
                nc.tensor.matmul(psI[:], lhsT=w2bd[:], rhs=hh[:, sl],
                                 start=True, stop=True)
                nc.scalar.activation(e4[:, sl], psI[:], AF.Exp,
                                     bias=nb2i[:], scale=-1.0)
                nc.scalar.activation(e4[:, sl], e4[:, sl], AF.Identity,
                                     bias=one4[:])
                nc.vector.reciprocal(e4[:, sl], e4[:, sl])
                nc.vector.tensor_tensor(e4[:, sl], e4[:, sl], qk[:, sl],
                                        op=OP.mult)
            # scatter head rows into estage3 [(32 i) p, 64 j @ hh*64]
            for cc in range(4):
                nc.scalar.dma_start(
                    estage3[:, (4 * g + cc) * NB:(4 * g + cc + 1) * NB],
                    e4[cc:cc + 1, :])
            # incremental (hh,j)->(j,hh) transpose for this grp's columns
            nc.scalar.copy(
                estage4[:].rearrange("p (j hh) -> p hh j", j=NB, hh=16)
                [:, 4 * g:4 * g + 4, :],
                estage3[:, 4 * g * NB:(4 * g + 4) * NB]
                .rearrange("p (hh j) -> p hh j", hh=4, j=NB))

        # ---- emit program ----
        for h in range(H):
            pool_dve(ks, h, bigk, kPack, "k")
        nc.gpsimd.dma_start(cc_in[:], kPack[:])
        nc.gpsimd.collective_compute(
            "AllGather", mybir.AluOpType.bypass,
            replica_groups=[[0, 1], [2, 3], [4, 5], [6, 7]],
            ins=[cc_in.opt()], outs=[cc_out.opt()])
        nc.gpsimd.dma_start(ktmp[:, 0:H * NBH], cc_out[0:128])
        nc.gpsimd.dma_start(ktmp[:, H * NBH:], cc_out[128:256])
        # interleave on ACT: kTall[(h, half, j)] <- ktmp[(half, h, j)]
        nc.scalar.copy(
            kTall[:].rearrange("p (h c j) -> p c h j", h=H, c=2, j=NBH),
            ktmp[:].rearrange("p (c h j) -> p c h j", c=2, h=H, j=NBH))

        def q_grp(g):
            paths = [pool_dve, pool_pe, pool_pe, pool_dve] if g in (0, 3) \
                else [pool_pe, pool_dve, pool_pe, pool_pe]
            for cc in range(4):
                paths[cc](qs, 4 * g + cc, bigq, qTall, "q")

        def grp_compute(g):
            mlp(qTall, g * 128, (g + 1) * 128, qimp[0:1, g * 128:(g + 1) * 128])
            mlp(kTall, g * 256, (g + 1) * 256, kimp[0:1, g * 256:(g + 1) * 256])
            sigma_inplace(qimp[0:1, g * 128:(g + 1) * 128])
            sigma_inplace(kimp[0:1, g * 256:(g + 1) * 256])
            interact_grp(g)

        q_grp(0)
        q_grp(1)
        grp_compute(0)
        q_grp(2)
        grp_compute(1)
        q_grp(3)
        grp_compute(2)
        grp_compute(3)

        # fold into foldedT [p=(r,a,jq), (jr,hh)], j = jq*16 + jr
        for rr in range(NROW):
            eng = nc.sync if rr % 2 == 0 else nc.scalar
            eng.dma_start(
                foldedT[16 * rr:16 * rr + 16, :]
                .rearrange("p (jr hh) -> p jr hh", jr=16, hh=16),
                estage4[4 * rr:4 * rr + 4, :]
                .rearrange("p (jq jr hh) -> p jq jr hh", jq=4, jr=16, hh=16))

        # ---- top-k: two-phase exact fp32 bisection (transposed layout) ----
        thr = persist.tile([128, 1], f32, tag="thr")
        pr = persist.tile([128, 1], f32, tag="pr")
        lo = persist.tile([128, 1], f32, tag="lo")
        ge = persist.tile([128, 256], f32, tag="ge")
        cntp = persist.tile([128, 1], f32, tag="cntp")

        def bisect(vals, nit):
            nc.vector.memset(thr[:], 0.5)
            s = 0.25
            for _ in range(nit):
                psC = small_ps.tile([128, 1], f32, tag="mlp")
                nc.vector.tensor_tensor(
                    ge[:], vals[:], thr[:].broadcast_to((128, 256)),
                    op=OP.is_ge)
                nc.vector.tensor_reduce(cntp[:], ge[:], axis=AX.X, op=OP.add)
                nc.tensor.matmul(psC[:], lhsT=blk16[:], rhs=cntp[:],
                                 start=True, stop=True)
                # pr = (cnt >= K) * 2s ; thr += pr - s
                nc.vector.tensor_scalar(pr[:], psC[:], float(KSEL), 2.0 * s,
                                        op0=OP.is_ge, op1=OP.mult)
                nc.vector.scalar_tensor_tensor(thr[:], pr[:], -s, thr[:],
                                               op0=OP.add, op1=OP.add)
                s *= 0.5
            return s

        s1 = bisect(foldedT, NIT1)
        nc.vector.tensor_scalar_add(lo[:], thr[:], -2.0 * s1)
        nc.vector.tensor_scalar(foldedT2[:], foldedT[:], lo[:],
                                float(2 ** NIT1), op0=OP.subtract, op1=OP.mult)
        s2 = bisect(foldedT2, NIT2)
        nc.vector.tensor_scalar_add(lo[:], thr[:], -2.0 * s2)

        mask = persist.tile([128, 256], u8, tag="mask")
        nc.vector.tensor_scalar(mask[:], foldedT2[:], lo[:], None, op0=OP.is_ge)
        nc.sync.dma_start(
            y[:].rearrange("r (a jq jr4) (jrm hh) -> (r a jq) jr4 jrm hh",
                           a=4, jq=4, jr4=4, jrm=4, hh=16),
            mask[:].rearrange("p (jr4 jrm hh) -> p jr4 jrm hh",
                              jr4=4, jrm=4, hh=16))

    nc.compile()
    return nc


def _constants(w_imp1, b_imp1, w_imp2, b_imp2, w_imp3, b_imp3,
               w_int1, b_int1, w_int2, b_int2):
    f = np.float32
    blkS = np.zeros((128, NBH), f)
    blkS[np.arange(128), np.arange(128) // 4] = 1.0 / 128.0
    blkq = np.zeros((128, 32), f)
    blkq[np.arange(128), np.arange(128) // 4] = 1.0
    blk16 = np.zeros((128, 128), f)
    blk16[np.arange(128)[:, None] // 16 == np.arange(128)[None, :] // 16] = 1.0
    consts = {
        "c_blkS": blkS,
        "c_blkq": blkq,
        "c_id32": (np.eye(32, dtype=f) / 128.0).astype(f),
        "c_blk16": blk16,
        "c_w1": np.ascontiguousarray(w_imp1, f),
        "c_b1": np.ascontiguousarray(np.asarray(b_imp1, f).reshape(HID1, 1)),
        "c_w2": np.ascontiguousarray(w_imp2, f),
        "c_b2": np.ascontiguousarray(np.asarray(b_imp2, f).reshape(HID2, 1)),
        "c_w3": np.ascontiguousarray(w_imp3, f),
        "c_nb3": np.ascontiguousarray(-np.asarray(b_imp3, f).reshape(1, 1)),
        "c_w1a": np.ascontiguousarray(np.asarray(w_int1, f)[:D]),
        "c_w1b": np.ascontiguousarray(np.asarray(w_int1, f)[D:]),
        "c_b1i": np.ascontiguousarray(
            np.tile(np.asarray(b_int1, f).reshape(HID1, 1), (4, 1))),
        "c_nb2i": np.ascontiguousarray(
            np.tile(-np.asarray(b_int2, f).reshape(1, 1), (4, 1))),
        "c_one4": np.ones((4, 1), f),
    }
    w2bd = np.zeros((128, 4), f)
    for c in range(4):
        w2bd[32 * c:32 * c + 32, c] = np.asarray(w_int2, f)[:, 0]
    consts["c_w2bd"] = w2bd
    return consts


def _in_maps(q, k, w_imp1, b_imp1, w_imp2, b_imp2, w_imp3, b_imp3,
             w_int1, b_int1, w_int2, b_int2):
    q = np.asarray(q, np.float32)
    k = np.asarray(k, np.float32)
    consts = _constants(w_imp1, b_imp1, w_imp2, b_imp2, w_imp3, b_imp3,
                        w_int1, b_int1, w_int2, b_int2)
    in_maps = []
    for c in range(NCORES):
        b, p = c // 2, c % 2
        m = {"qs": q[b, p * SH:(p + 1) * SH],
             "ks": k[b, p * SH:(p + 1) * SH]}
        m.update(consts)
        in_maps.append(m)
    return in_maps


def kernel(q, k, **w):
    from concourse.bass_utils import run_bass_kernel_spmd

    in_maps = _in_maps(q, k, **w)

    if "nc" not in _nc_cache:
        _nc_cache["nc"] = _build_nc()
    res = run_bass_kernel_spmd(_nc_cache["nc"], in_maps,
                               core_ids=list(range(NCORES)))
    out = np.empty((B, H, NB, NB), np.uint8)
    for c in range(NCORES):
        b, p = c // 2, c % 2
        out[b, p * 8:(p + 1) * 8] = res.results[c]["y"]
    return out > 0


# revision 26
# speedup vs baseline: 1.5980x; 1.0203x over previous
"""Trainium2 Bass kernel for ContentAdaptiveSparsity (topk_masking).

Reference semantics: combined[b,i,j,h] = q_imp[b,i,h] * k_imp[b,j,h] *
interaction[b,i,j,h] from block-mean pooled q,k (64 blocks of 128) through
tiny MLPs; raw row-major reshape [B,nb,nb,H] -> [B,16,4096]; top-1024 mask
per (b, r) row, candidate m = (i%4)*1024 + j*16 + h, out y[r, m//64, m%64].

Sharding: core c = (batch b=c//2, seq half p=c%2) reads the CONTIGUOUS
slices q[b, p*4096:...] / k[b, p*4096:...] (64 MB, zero-copy host views).
Its 32 i-blocks are exactly its 8 output rows r in [8p, 8p+8); its 32
j-blocks are half of k.  Core pairs exchange pooled-k halves (256 KB) via
pairwise AllGather through DRAM.

Device pipeline:
  - 32 per-head tile loads [128 chunks x (32 rows, 128 d)] on SP, gather
    APs straight off the raw [S,H,D] layout (512B descriptors).
  - pooling split across engines: k heads + first q heads on DVE (grouped
    free-dim reduce -> [128a,128d] partial, then one matmul lhsT=partial,
    rhs=chunk->block/128 that fuses cross-chunk sum + mean + transpose);
    remaining q heads on PE (block-diag ones matmuls accumulating
    [32,512] psum + small DVE grouped reduce + id32/128 transpose).
  - k first -> kPack -> DRAM -> AllGather(pair) -> kTall [128,(h,64)];
    per 4-head grp: importance MLPs, qk=qimp x kimp outer (off critical
    path), interaction grid add (DVE), relu (ACT), block-diag w2 matmul
    per 512-col slice pipelined with exp (ACT), +1 (ACT bias), recip
    (DVE), * qk (DVE); scatter rows to estage3, per-grp transpose copy
    (hh,j)->(j,hh) into estage4 (ACT).
  - fold: 8 DMAs estage4 -> foldedT [128=(r,a,jq), (jr,hh)] (1KB descs):
    row r occupies partitions 16r..16r+16; its 4096 candidates are
    (a,jq | jr,hh) with j = jq*16+jr.
  - top-k: two-phase exact fp32 bisection from (0,1], compile-time
    power-of-2 steps (thr sums EXACT in fp32).  Per iter: DVE compare +
    per-partition count, blk16 block-diag matmul summing each row's 16
    partitions (result replicated across the group), then two fused
    threshold-update ops (tensor_scalar is_ge*2s, scalar_tensor_tensor).
    After 17 iters bracket width 2^-17; rebase foldedT2 =
    (foldedT - L)*2^17 in ONE fused op (Sterbenz-exact near the bracket),
    11 more iters -> resolution 2^-28, well under the min candidate gap
    (~1.5e-8).  Mask = (foldedT2 >= L2) via per-partition scalar compare.
"""

import numpy as np

B, S, H, D = 4, 8192, 16, 128
SH = S // 2       # per-core seq slice
NB = 64           # blocks per sequence
NBH = 32          # blocks per core half
NROW = 8          # topk rows (r) per core
NCORES = 8
KSEL = 1024
HID1, HID2 = 32, 16
NIT1, NIT2 = 17, 11
N_Q_DVE = 4       # q heads pooled on the DVE path (rest on PE)

_nc_cache = {}


def _build_nc():
    from contextlib import ExitStack

    from concourse import bacc
    import concourse.mybir as mybir
    from concourse.tile import TileContext

    f32 = mybir.dt.float32
    u8 = mybir.dt.uint8
    AF = mybir.ActivationFunctionType
    OP = mybir.AluOpType
    AX = mybir.AxisListType

    nc = bacc.Bacc("TRN2", target_bir_lowering=False, debug=False,
                   num_devices=NCORES)

    qs = nc.dram_tensor("qs", [SH, H, D], f32, kind="ExternalInput")
    ks = nc.dram_tensor("ks", [SH, H, D], f32, kind="ExternalInput")
    c_blkS = nc.dram_tensor("c_blkS", [128, NBH], f32, kind="ExternalInput")
    c_blkq = nc.dram_tensor("c_blkq", [128, 32], f32, kind="ExternalInput")
    c_id32 = nc.dram_tensor("c_id32", [32, 32], f32, kind="ExternalInput")
    c_blk16 = nc.dram_tensor("c_blk16", [128, 128], f32, kind="ExternalInput")
    c_w1 = nc.dram_tensor("c_w1", [D, HID1], f32, kind="ExternalInput")
    c_b1 = nc.dram_tensor("c_b1", [HID1, 1], f32, kind="ExternalInput")
    c_w2 = nc.dram_tensor("c_w2", [HID1, HID2], f32, kind="ExternalInput")
    c_b2 = nc.dram_tensor("c_b2", [HID2, 1], f32, kind="ExternalInput")
    c_w3 = nc.dram_tensor("c_w3", [HID2, 1], f32, kind="ExternalInput")
    c_nb3 = nc.dram_tensor("c_nb3", [1, 1], f32, kind="ExternalInput")
    c_w1a = nc.dram_tensor("c_w1a", [D, HID1], f32, kind="ExternalInput")
    c_w1b = nc.dram_tensor("c_w1b", [D, HID1], f32, kind="ExternalInput")
    c_b1i = nc.dram_tensor("c_b1i", [128, 1], f32, kind="ExternalInput")
    c_w2bd = nc.dram_tensor("c_w2bd", [128, 4], f32, kind="ExternalInput")
    c_nb2i = nc.dram_tensor("c_nb2i", [4, 1], f32, kind="ExternalInput")
    c_one4 = nc.dram_tensor("c_one4", [4, 1], f32, kind="ExternalInput")

    y = nc.dram_tensor("y", [NROW, NB, NB], u8, kind="ExternalOutput")

    with TileContext(nc) as tc, ExitStack() as ctx:
        const = ctx.enter_context(tc.tile_pool(name="const", bufs=1))
        bigk = ctx.enter_context(tc.tile_pool(name="bigk", bufs=3))
        bigq = ctx.enter_context(tc.tile_pool(name="bigq", bufs=4))
        part_pool = ctx.enter_context(tc.tile_pool(name="part", bufs=3))
        hpool = ctx.enter_context(tc.tile_pool(name="hpool", bufs=2))
        sb = ctx.enter_context(tc.tile_pool(name="sb", bufs=2))
        persist = ctx.enter_context(tc.tile_pool(name="persist", bufs=1))
        dram = ctx.enter_context(tc.tile_pool(name="dram", bufs=1, space="DRAM"))
        pool_ps = ctx.enter_context(tc.tile_pool(name="pool_ps", bufs=1, space="PSUM"))
        pool2_ps = ctx.enter_context(tc.tile_pool(name="pool2_ps", bufs=2, space="PSUM"))
        tp_ps = ctx.enter_context(tc.tile_pool(name="tp_ps", bufs=2, space="PSUM"))
        small_ps = ctx.enter_context(tc.tile_pool(name="small_ps", bufs=1, space="PSUM"))
        int_ps = ctx.enter_context(tc.tile_pool(name="int_ps", bufs=2, space="PSUM"))

        def cload(dramt, shape, tag):
            t = const.tile(shape, f32, tag=tag)
            nc.gpsimd.dma_start(t[:], dramt[:])
            return t

        blkS = cload(c_blkS, [128, NBH], "blkS")
        blkq = cload(c_blkq, [128, 32], "blkq")
        id32 = cload(c_id32, [32, 32], "id32")
        blk16 = cload(c_blk16, [128, 128], "blk16")
        w1 = cload(c_w1, [D, HID1], "w1")
        b1 = cload(c_b1, [HID1, 1], "b1")
        w2 = cload(c_w2, [HID1, HID2], "w2")
        b2 = cload(c_b2, [HID2, 1], "b2")
        w3 = cload(c_w3, [HID2, 1], "w3")
        nb3 = cload(c_nb3, [1, 1], "nb3")
        w1a = cload(c_w1a, [D, HID1], "w1a")
        w1b = cload(c_w1b, [D, HID1], "w1b")
        b1i = cload(c_b1i, [128, 1], "b1i")
        w2bd = cload(c_w2bd, [128, 4], "w2bd")
        nb2i = cload(c_nb2i, [4, 1], "nb2i")
        one4 = cload(c_one4, [4, 1], "one4")

        qTall = persist.tile([128, H * NBH], f32, tag="qTall")    # (h, i32)
        kPack = persist.tile([128, H * NBH], f32, tag="kPack")    # (h, j32) mine
        kTall = persist.tile([128, H * NB], f32, tag="kTall")     # (h, j64)
        ktmp = persist.tile([128, H * NB], f32, tag="ktmp")       # (half, h, j32)
        qimp = persist.tile([1, H * NBH], f32, tag="qimp")
        kimp = persist.tile([1, H * NB], f32, tag="kimp")
        estage3 = persist.tile([NBH, H * NB], f32, tag="estage3")  # [i, (hh,j)]
        estage4 = persist.tile([NBH, H * NB], f32, tag="estage4")  # [i, (j,hh)]
        foldedT = persist.tile([128, 256], f32, tag="foldedT")
        foldedT2 = persist.tile([128, 256], f32, tag="foldedT2")

        cc_in = dram.tile([128, H * NBH], f32)
        cc_out = dram.tile([2 * 128, H * NBH], f32)

        def pool_dve(src, h, big, dst, dtag):
            """DVE-path block-mean pool of head h -> dst cols [h*32,(h+1)*32)."""
            xt = big.tile([128, 4096], f32, tag=dtag)
            nc.sync.dma_start(
                xt[:].rearrange("a (b d) -> a b d", b=32),
                src[:, h, :].rearrange("(a b) d -> a b d", a=128))
            part = part_pool.tile([128, 128], f32, tag="part")
            nc.vector.tensor_reduce(
                part[:], xt[:].rearrange("a (b d) -> a d b", b=32),
                axis=AX.X, op=OP.add)
            ps = pool_ps.tile([128, NBH], f32, tag="pool")
            nc.tensor.matmul(ps[:], lhsT=part[:], rhs=blkS[:],
                             start=True, stop=True)
            nc.scalar.copy(dst[:, h * NBH:(h + 1) * NBH], ps[:])

        def pool_pe(src, h, big, dst, dtag):
            """PE-path block-mean pool of head h -> dst cols [h*32,(h+1)*32)."""
            xt = big.tile([128, 4096], f32, tag=dtag)
            nc.sync.dma_start(
                xt[:].rearrange("a (b d) -> a b d", b=32),
                src[:, h, :].rearrange("(a b) d -> a b d", a=128))
            ps = pool2_ps.tile([32, 512], f32, tag="pool2")
            for j in range(8):
                nc.tensor.matmul(ps[:], lhsT=blkq[:], rhs=xt[:, j * 512:(j + 1) * 512],
                                 start=(j == 0), stop=(j == 7))
            sbp = sb.tile([32, 128], f32, tag="sbp")
            nc.vector.tensor_reduce(
                sbp[:], ps[:].rearrange("m (s d) -> m d s", s=4, d=128),
                axis=AX.X, op=OP.add)
            pst = tp_ps.tile([128, NBH], f32, tag="tp")
            nc.tensor.matmul(pst[:], lhsT=sbp[:], rhs=id32[:], start=True, stop=True)
            nc.scalar.copy(dst[:, h * NBH:(h + 1) * NBH], pst[:])

        def mlp(xT, n0, n1, imp_dst):
            """importance MLP on xT cols [n0:n1] -> imp_dst (exp(-logit))."""
            n = n1 - n0
            ps1 = small_ps.tile([HID1, n], f32, tag="mlp")
            nc.tensor.matmul(ps1[:], lhsT=w1[:], rhs=xT[:, n0:n1], start=True, stop=True)
            h1 = sb.tile([HID1, n], f32, tag="h1")
            nc.scalar.activation(h1[:], ps1[:], AF.Relu, bias=b1[:])
            ps2 = small_ps.tile([HID2, n], f32, tag="mlp")
            nc.tensor.matmul(ps2[:], lhsT=w2[:], rhs=h1[:], start=True, stop=True)
            h2 = sb.tile([HID2, n], f32, tag="h2")
            nc.scalar.activation(h2[:], ps2[:], AF.Relu, bias=b2[:])
            ps3 = small_ps.tile([1, n], f32, tag="mlp")
            nc.tensor.matmul(ps3[:], lhsT=w3[:], rhs=h2[:], start=True, stop=True)
            nc.scalar.activation(imp_dst, ps3[:], AF.Exp, bias=nb3[:], scale=-1.0)

        def sigma_inplace(ap):
            nc.vector.tensor_scalar_add(ap, ap, 1.0)
            nc.vector.reciprocal(ap, ap)

        def interact_grp(g):
            """heads hh=4g..4g+3: interaction + combine -> estage3/estage4."""
            # qk = qimp (x) kimp outer product, off the critical path
            qg = sb.tile([4, NBH], f32, tag="qg")
            nc.gpsimd.dma_start(
                qg[:], qimp[0:1, g * 128:(g + 1) * 128]
                .rearrange("o (c i) -> o c i", c=4))
            kg = sb.tile([4, NB], f32, tag="kg")
            nc.gpsimd.dma_start(
                kg[:], kimp[0:1, g * 256:(g + 1) * 256]
                .rearrange("o (c j) -> o c j", c=4))
            qk = sb.tile([4, NBH * NB], f32, tag="qk")
            nc.vector.tensor_tensor(
                qk[:].rearrange("p (i j) -> p i j", i=NBH),
                qg[:].unsqueeze(2).broadcast_to((4, NBH, NB)),
                kg[:].unsqueeze(1).broadcast_to((4, NBH, NB)),
                op=OP.mult)

            psq = tp_ps.tile([128, NBH], f32, tag="tp")
            for cc in range(4):
                nc.tensor.matmul(psq[32 * cc:32 * cc + 32, :], lhsT=w1a[:],
                                 rhs=qTall[:, (4 * g + cc) * NBH:(4 * g + cc + 1) * NBH],
                                 tile_position=(0, 32 * cc), start=True, stop=True)
            qp4 = sb.tile([128, NBH], f32, tag="qp4")
            nc.scalar.copy(qp4[:], psq[:])
            psk = tp_ps.tile([128, NB], f32, tag="tp")
            for cc in range(4):
                nc.tensor.matmul(psk[32 * cc:32 * cc + 32, :], lhsT=w1b[:],
                                 rhs=kTall[:, (4 * g + cc) * NB:(4 * g + cc + 1) * NB],
                                 tile_position=(0, 32 * cc), start=True, stop=True)
            kp4 = sb.tile([128, NB], f32, tag="kp4")
            nc.scalar.activation(kp4[:], psk[:], AF.Identity, bias=b1i[:])
            hh = hpool.tile([128, NBH * NB], f32, tag="hh")
            nc.vector.tensor_tensor(
                hh[:].rearrange("p (i j) -> p i j", i=NBH),
                qp4[:].unsqueeze(2).broadcast_to((128, NBH, NB)),
                kp4[:].unsqueeze(1).broadcast_to((128, NBH, NB)),
                op=OP.add)
            nc.scalar.activation(hh[:], hh[:], AF.Relu)
            e4 = sb.tile([4, NBH * NB], f32, tag="e4")
            for n in range(4):
                sl = slice(n * 512, (n + 1) * 512)
                psI = int_ps.tile([4, 512], f32, tag="int")
                nc.tensor.matmul(psI[:], lhsT=w2bd[:], rhs=hh[:, sl],
                                 start=True, stop=True)
                nc.scalar.activation(e4[:, sl], psI[:], AF.Exp,
                                     bias=nb2i[:], scale=-1.0)
                nc.scalar.activation(e4[:, sl], e4[:, sl], AF.Identity,
                                     bias=one4[:])
                nc.vector.reciprocal(e4[:, sl], e4[:, sl])
                nc.vector.tensor_tensor(e4[:, sl], e4[:, sl], qk[:, sl],
                                        op=OP.mult)
            # scatter head rows into estage3 [(32 i) p, 64 j @ hh*64]
            for cc in range(4):
                nc.scalar.dma_start(
                    estage3[:, (4 * g + cc) * NB:(4 * g + cc + 1) * NB],
                    e4[cc:cc + 1, :])
            # incremental (hh,j)->(j,hh) transpose for this grp's columns
            nc.scalar.copy(
                estage4[:].rearrange("p (j hh) -> p hh j", j=NB, hh=16)
                [:, 4 * g:4 * g + 4, :],
                estage3[:, 4 * g * NB:(4 * g + 4) * NB]
                .rearrange("p (hh j) -> p hh j", hh=4, j=NB))

        # ---- emit program ----
        for h in range(H):
            pool_dve(ks, h, bigk, kPack, "k")
        nc.gpsimd.dma_start(cc_in[:], kPack[:])
        nc.gpsimd.collective_compute(
            "AllGather", mybir.AluOpType.bypass,
            replica_groups=[[0, 1], [2, 3], [4, 5], [6, 7]],
            ins=[cc_in.opt()], outs=[cc_out.opt()])
        nc.gpsimd.dma_start(ktmp[:, 0:H * NBH], cc_out[0:128])
        nc.gpsimd.dma_start(ktmp[:, H * NBH:], cc_out[128:256])
        # interleave on ACT: kTall[(h, half, j)] <- ktmp[(half, h, j)]
        nc.scalar.copy(
            kTall[:].rearrange("p (h c j) -> p c h j", h=H, c=2, j=NBH),
            ktmp[:].rearrange("p (c h j) -> p c h j", c=2, h=H, j=NBH))

        def q_grp(g):
            paths = [pool_dve, pool_pe, pool_pe, pool_dve] if g in (0, 3) \
                else [pool_pe, pool_dve, pool_pe, pool_pe]
            for cc in range(4):
                paths[cc](qs, 4 * g + cc, bigq, qTall, "q")

        def grp_compute(g):
            mlp(qTall, g * 128, (g + 1) * 128, qimp[0:1, g * 128:(g + 1) * 128])
            mlp(kTall, g * 256, (g + 1) * 256, kimp[0:1, g * 256:(g + 1) * 256])
            sigma_inplace(qimp[0:1, g * 128:(g + 1) * 128])
            sigma_inplace(kimp[0:1, g * 256:(g + 1) * 256])
            interact_grp(g)

        q_grp(0)
        q_grp(1)
        grp_compute(0)
        q_grp(2)
        grp_compute(1)
        grp_compute(2)
        q_grp(3)
        grp_compute(3)

        # fold into foldedT [p=(r,a,jq), (jr,hh)], j = jq*16 + jr
        for rr in range(NROW):
            eng = nc.sync if rr % 2 == 0 else nc.scalar
            eng.dma_start(
                foldedT[16 * rr:16 * rr + 16, :]
                .rearrange("p (jr hh) -> p jr hh", jr=16, hh=16),
                estage4[4 * rr:4 * rr + 4, :]
                .rearrange("p (jq jr hh) -> p jq jr hh", jq=4, jr=16, hh=16))

        # ---- top-k: two-phase exact fp32 bisection (transposed layout) ----
        thr = persist.tile([128, 1], f32, tag="thr")
        pr = persist.tile([128, 1], f32, tag="pr")
        lo = persist.tile([128, 1], f32, tag="lo")
        ge = persist.tile([128, 256], f32, tag="ge")
        cntp = persist.tile([128, 1], f32, tag="cntp")

        def bisect(vals, nit):
            nc.vector.memset(thr[:], 0.5)
            s = 0.25
            for _ in range(nit):
                psC = small_ps.tile([128, 1], f32, tag="mlp")
                nc.vector.tensor_tensor(
                    ge[:], vals[:], thr[:].broadcast_to((128, 256)),
                    op=OP.is_ge)
                nc.vector.tensor_reduce(cntp[:], ge[:], axis=AX.X, op=OP.add)
                nc.tensor.matmul(psC[:], lhsT=blk16[:], rhs=cntp[:],
                                 start=True, stop=True)
                # pr = (cnt >= K) * 2s ; thr += pr - s
                nc.vector.tensor_scalar(pr[:], psC[:], float(KSEL), 2.0 * s,
                                        op0=OP.is_ge, op1=OP.mult)
                nc.vector.scalar_tensor_tensor(thr[:], pr[:], -s, thr[:],
                                               op0=OP.add, op1=OP.add)
                s *= 0.5
            return s

        s1 = bisect(foldedT, NIT1)
        nc.vector.tensor_scalar_add(lo[:], thr[:], -2.0 * s1)
        nc.vector.tensor_scalar(foldedT2[:], foldedT[:], lo[:],
                                float(2 ** NIT1), op0=OP.subtract, op1=OP.mult)
        s2 = bisect(foldedT2, NIT2)
        nc.vector.tensor_scalar_add(lo[:], thr[:], -2.0 * s2)

        mask = persist.tile([128, 256], u8, tag="mask")
        nc.vector.tensor_scalar(mask[:], foldedT2[:], lo[:], None, op0=OP.is_ge)
        nc.sync.dma_start(
            y[:].rearrange("r (a jq jr4) (jrm hh) -> (r a jq) jr4 jrm hh",
                           a=4, jq=4, jr4=4, jrm=4, hh=16),
            mask[:].rearrange("p (jr4 jrm hh) -> p jr4 jrm hh",
                              jr4=4, jrm=4, hh=16))

    nc.compile()
    return nc


def _constants(w_imp1, b_imp1, w_imp2, b_imp2, w_imp3, b_imp3,
               w_int1, b_int1, w_int2, b_int2):
    f = np.float32
    blkS = np.zeros((128, NBH), f)
    blkS[np.arange(128), np.arange(128) // 4] = 1.0 / 128.0
    blkq = np.zeros((128, 32), f)
    blkq[np.arange(128), np.arange(128) // 4] = 1.0
    blk16 = np.zeros((128, 128), f)
    blk16[np.arange(128)[:, None] // 16 == np.arange(128)[None, :] // 16] = 1.0
    consts = {
        "c_blkS": blkS,
        "c_blkq": blkq,
        "c_id32": (np.eye(32, dtype=f) / 128.0).astype(f),
        "c_blk16": blk16,
        "c_w1": np.ascontiguousarray(w_imp1, f),
        "c_b1": np.ascontiguousarray(np.asarray(b_imp1, f).reshape(HID1, 1)),
        "c_w2": np.ascontiguousarray(w_imp2, f),
        "c_b2": np.ascontiguousarray(np.asarray(b_imp2, f).reshape(HID2, 1)),
        "c_w3": np.ascontiguousarray(w_imp3, f),
        "c_nb3": np.ascontiguousarray(-np.asarray(b_imp3, f).reshape(1, 1)),
        "c_w1a": np.ascontiguousarray(np.asarray(w_int1, f)[:D]),
        "c_w1b": np.ascontiguousarray(np.asarray(w_int1, f)[D:]),
        "c_b1i": np.ascontiguousarray(
            np.tile(np.asarray(b_int1, f).reshape(HID1, 1), (4, 1))),
        "c_nb2i": np.ascontiguousarray(
            np.tile(-np.asarray(b_int2, f).reshape(1, 1), (4, 1))),
        "c_one4": np.ones((4, 1), f),
    }
    w2bd = np.zeros((128, 4), f)
    for c in range(4):
        w2bd[32 * c:32 * c + 32, c] = np.asarray(w_int2, f)[:, 0]
    consts["c_w2bd"] = w2bd
    return consts


def _in_maps(q, k, w_imp1, b_imp1, w_imp2, b_imp2, w_imp3, b_imp3,
             w_int1, b_int1, w_int2, b_int2):
    q = np.asarray(q, np.float32)
    k = np.asarray(k, np.float32)
    consts = _constants(w_imp1, b_imp1, w_imp2, b_imp2, w_imp3, b_imp3,
                        w_int1, b_int1, w_int2, b_int2)
    in_maps = []
    for c in range(NCORES):
        b, p = c // 2, c % 2
        m = {"qs": q[b, p * SH:(p + 1) * SH],
             "ks": k[b, p * SH:(p + 1) * SH]}
        m.update(consts)
        in_maps.append(m)
    return in_maps


def kernel(q, k, **w):
    from concourse.bass_utils import run_bass_kernel_spmd

    in_maps = _in_maps(q, k, **w)

    if "nc" not in _nc_cache:
        _nc_cache["nc"] = _build_nc()
    res = run_bass_kernel_spmd(_nc_cache["nc"], in_maps,
                               core_ids=list(range(NCORES)))
    out = np.empty((B, H, NB, NB), np.uint8)
    for c in range(NCORES):
        b, p = c // 2, c % 2
        out[b, p * 8:(p + 1) * 8] = res.results[c]["y"]
    return out > 0


# revision 33
# speedup vs baseline: 1.6039x; 1.0037x over previous
"""Trainium2 Bass kernel for ContentAdaptiveSparsity (topk_masking).

Reference semantics: combined[b,i,j,h] = q_imp[b,i,h] * k_imp[b,j,h] *
interaction[b,i,j,h] from block-mean pooled q,k (64 blocks of 128) through
tiny MLPs; raw row-major reshape [B,nb,nb,H] -> [B,16,4096]; top-1024 mask
per (b, r) row, candidate m = (i%4)*1024 + j*16 + h, out y[r, m//64, m%64].

Sharding: core c = (batch b=c//2, seq half p=c%2) reads the CONTIGUOUS
slices q[b, p*4096:...] / k[b, p*4096:...] (64 MB, zero-copy host views).
Its 32 i-blocks are exactly its 8 output rows r in [8p, 8p+8); its 32
j-blocks are half of k.  Core pairs exchange pooled-k halves (256 KB) via
pairwise AllGather through DRAM.

Device pipeline:
  - 32 per-head tile loads [128 chunks x (32 rows, 128 d)] on SP, gather
    APs straight off the raw [S,H,D] layout (512B descriptors).
  - pooling split across engines: k heads + first q heads on DVE (grouped
    free-dim reduce -> [128a,128d] partial, then one matmul lhsT=partial,
    rhs=chunk->block/128 that fuses cross-chunk sum + mean + transpose);
    remaining q heads on PE (block-diag ones matmuls accumulating
    [32,512] psum + small DVE grouped reduce + id32/128 transpose).
  - k first -> kPack -> DRAM -> AllGather(pair) -> kTall [128,(h,64)];
    per 4-head grp: importance MLPs, qk=qimp x kimp outer (off critical
    path), interaction grid add (DVE), relu (ACT), block-diag w2 matmul
    per 512-col slice pipelined with exp (ACT), +1 (ACT bias), recip
    (DVE), * qk (DVE); scatter rows to estage3, per-grp transpose copy
    (hh,j)->(j,hh) into estage4 (ACT).
  - fold: 8 DMAs estage4 -> foldedT [128=(r,a,jq), (jr,hh)] (1KB descs):
    row r occupies partitions 16r..16r+16; its 4096 candidates are
    (a,jq | jr,hh) with j = jq*16+jr.
  - top-k: two-phase exact fp32 bisection from (0,1], compile-time
    power-of-2 steps (thr sums EXACT in fp32).  Per iter: DVE compare +
    per-partition count, blk16 block-diag matmul summing each row's 16
    partitions (result replicated across the group), then two fused
    threshold-update ops (tensor_scalar is_ge*2s, scalar_tensor_tensor).
    After 17 iters bracket width 2^-17; rebase foldedT2 =
    (foldedT - L)*2^17 in ONE fused op (Sterbenz-exact near the bracket),
    11 more iters -> resolution 2^-28, well under the min candidate gap
    (~1.5e-8).  Mask = (foldedT2 >= L2) via per-partition scalar compare.
"""

import numpy as np

B, S, H, D = 4, 8192, 16, 128
SH = S // 2       # per-core seq slice
NB = 64           # blocks per sequence
NBH = 32          # blocks per core half
NROW = 8          # topk rows (r) per core
NCORES = 8
KSEL = 1024
HID1, HID2 = 32, 16
NIT1, NIT2 = 17, 10
N_Q_DVE = 4       # q heads pooled on the DVE path (rest on PE)

_nc_cache = {}


def _build_nc():
    from contextlib import ExitStack

    from concourse import bacc
    import concourse.mybir as mybir
    from concourse.tile import TileContext

    f32 = mybir.dt.float32
    u8 = mybir.dt.uint8
    AF = mybir.ActivationFunctionType
    OP = mybir.AluOpType
    AX = mybir.AxisListType

    nc = bacc.Bacc("TRN2", target_bir_lowering=False, debug=False,
                   num_devices=NCORES)

    qs = nc.dram_tensor("qs", [SH, H, D], f32, kind="ExternalInput")
    ks = nc.dram_tensor("ks", [SH, H, D], f32, kind="ExternalInput")
    c_blkS = nc.dram_tensor("c_blkS", [128, NBH], f32, kind="ExternalInput")
    c_blkq = nc.dram_tensor("c_blkq", [128, 32], f32, kind="ExternalInput")
    c_id32 = nc.dram_tensor("c_id32", [32, 32], f32, kind="ExternalInput")
    c_blk16 = nc.dram_tensor("c_blk16", [128, 128], f32, kind="ExternalInput")
    c_w1 = nc.dram_tensor("c_w1", [D, HID1], f32, kind="ExternalInput")
    c_b1 = nc.dram_tensor("c_b1", [HID1, 1], f32, kind="ExternalInput")
    c_w2 = nc.dram_tensor("c_w2", [HID1, HID2], f32, kind="ExternalInput")
    c_b2 = nc.dram_tensor("c_b2", [HID2, 1], f32, kind="ExternalInput")
    c_w3 = nc.dram_tensor("c_w3", [HID2, 1], f32, kind="ExternalInput")
    c_nb3 = nc.dram_tensor("c_nb3", [1, 1], f32, kind="ExternalInput")
    c_w1a = nc.dram_tensor("c_w1a", [D, HID1], f32, kind="ExternalInput")
    c_w1b = nc.dram_tensor("c_w1b", [D, HID1], f32, kind="ExternalInput")
    c_b1i = nc.dram_tensor("c_b1i", [128, 1], f32, kind="ExternalInput")
    c_w2bd = nc.dram_tensor("c_w2bd", [128, 4], f32, kind="ExternalInput")
    c_nb2i = nc.dram_tensor("c_nb2i", [4, 1], f32, kind="ExternalInput")
    c_one4 = nc.dram_tensor("c_one4", [4, 1], f32, kind="ExternalInput")

    y = nc.dram_tensor("y", [NROW, NB, NB], u8, kind="ExternalOutput")

    with TileContext(nc) as tc, ExitStack() as ctx:
        const = ctx.enter_context(tc.tile_pool(name="const", bufs=1))
        bigk = ctx.enter_context(tc.tile_pool(name="bigk", bufs=3))
        bigq = ctx.enter_context(tc.tile_pool(name="bigq", bufs=4))
        part_pool = ctx.enter_context(tc.tile_pool(name="part", bufs=3))
        hpool = ctx.enter_context(tc.tile_pool(name="hpool", bufs=2))
        sb = ctx.enter_context(tc.tile_pool(name="sb", bufs=2))
        persist = ctx.enter_context(tc.tile_pool(name="persist", bufs=1))
        dram = ctx.enter_context(tc.tile_pool(name="dram", bufs=1, space="DRAM"))
        pool_ps = ctx.enter_context(tc.tile_pool(name="pool_ps", bufs=1, space="PSUM"))
        pool2_ps = ctx.enter_context(tc.tile_pool(name="pool2_ps", bufs=2, space="PSUM"))
        tp_ps = ctx.enter_context(tc.tile_pool(name="tp_ps", bufs=2, space="PSUM"))
        small_ps = ctx.enter_context(tc.tile_pool(name="small_ps", bufs=1, space="PSUM"))
        int_ps = ctx.enter_context(tc.tile_pool(name="int_ps", bufs=2, space="PSUM"))

        def cload(dramt, shape, tag):
            t = const.tile(shape, f32, tag=tag)
            nc.gpsimd.dma_start(t[:], dramt[:])
            return t

        blkS = cload(c_blkS, [128, NBH], "blkS")
        blkq = cload(c_blkq, [128, 32], "blkq")
        id32 = cload(c_id32, [32, 32], "id32")
        blk16 = cload(c_blk16, [128, 128], "blk16")
        w1 = cload(c_w1, [D, HID1], "w1")
        b1 = cload(c_b1, [HID1, 1], "b1")
        w2 = cload(c_w2, [HID1, HID2], "w2")
        b2 = cload(c_b2, [HID2, 1], "b2")
        w3 = cload(c_w3, [HID2, 1], "w3")
        nb3 = cload(c_nb3, [1, 1], "nb3")
        w1a = cload(c_w1a, [D, HID1], "w1a")
        w1b = cload(c_w1b, [D, HID1], "w1b")
        b1i = cload(c_b1i, [128, 1], "b1i")
        w2bd = cload(c_w2bd, [128, 4], "w2bd")
        nb2i = cload(c_nb2i, [4, 1], "nb2i")
        one4 = cload(c_one4, [4, 1], "one4")

        qTall = persist.tile([128, H * NBH], f32, tag="qTall")    # (h, i32)
        kPack = persist.tile([128, H * NBH], f32, tag="kPack")    # (h, j32) mine
        kTall = persist.tile([128, H * NB], f32, tag="kTall")     # (h, j64)
        ktmp = persist.tile([128, H * NB], f32, tag="ktmp")       # (half, h, j32)
        qimp = persist.tile([1, H * NBH], f32, tag="qimp")
        kimp = persist.tile([1, H * NB], f32, tag="kimp")
        estage3 = persist.tile([NBH, H * NB], f32, tag="estage3")  # [i, (hh,j)]
        estage4 = persist.tile([NBH, H * NB], f32, tag="estage4")  # [i, (j,hh)]
        foldedT = persist.tile([128, 256], f32, tag="foldedT")
        foldedT2 = persist.tile([128, 256], f32, tag="foldedT2")

        cc_in = dram.tile([128, H * NBH], f32)
        cc_out = dram.tile([2 * 128, H * NBH], f32)

        def pool_dve(src, h, big, dst, dtag):
            """DVE-path block-mean pool of head h -> dst cols [h*32,(h+1)*32)."""
            xt = big.tile([128, 4096], f32, tag=dtag)
            nc.sync.dma_start(
                xt[:].rearrange("a (b d) -> a b d", b=32),
                src[:, h, :].rearrange("(a b) d -> a b d", a=128))
            part = part_pool.tile([128, 128], f32, tag="part")
            nc.vector.tensor_reduce(
                part[:], xt[:].rearrange("a (b d) -> a d b", b=32),
                axis=AX.X, op=OP.add)
            ps = pool_ps.tile([128, NBH], f32, tag="pool")
            nc.tensor.matmul(ps[:], lhsT=part[:], rhs=blkS[:],
                             start=True, stop=True)
            nc.scalar.copy(dst[:, h * NBH:(h + 1) * NBH], ps[:])

        def pool_pe(src, h, big, dst, dtag):
            """PE-path block-mean pool of head h -> dst cols [h*32,(h+1)*32)."""
            xt = big.tile([128, 4096], f32, tag=dtag)
            nc.sync.dma_start(
                xt[:].rearrange("a (b d) -> a b d", b=32),
                src[:, h, :].rearrange("(a b) d -> a b d", a=128))
            ps = pool2_ps.tile([32, 512], f32, tag="pool2")
            for j in range(8):
                nc.tensor.matmul(ps[:], lhsT=blkq[:], rhs=xt[:, j * 512:(j + 1) * 512],
                                 start=(j == 0), stop=(j == 7))
            sbp = sb.tile([32, 128], f32, tag="sbp")
            nc.vector.tensor_reduce(
                sbp[:], ps[:].rearrange("m (s d) -> m d s", s=4, d=128),
                axis=AX.X, op=OP.add)
            pst = tp_ps.tile([128, NBH], f32, tag="tp")
            nc.tensor.matmul(pst[:], lhsT=sbp[:], rhs=id32[:], start=True, stop=True)
            nc.scalar.copy(dst[:, h * NBH:(h + 1) * NBH], pst[:])

        def mlp(xT, n0, n1, imp_dst):
            """importance MLP on xT cols [n0:n1] -> imp_dst (exp(-logit))."""
            n = n1 - n0
            ps1 = small_ps.tile([HID1, n], f32, tag="mlp")
            nc.tensor.matmul(ps1[:], lhsT=w1[:], rhs=xT[:, n0:n1], start=True, stop=True)
            h1 = sb.tile([HID1, n], f32, tag="h1")
            nc.scalar.activation(h1[:], ps1[:], AF.Relu, bias=b1[:])
            ps2 = small_ps.tile([HID2, n], f32, tag="mlp")
            nc.tensor.matmul(ps2[:], lhsT=w2[:], rhs=h1[:], start=True, stop=True)
            h2 = sb.tile([HID2, n], f32, tag="h2")
            nc.scalar.activation(h2[:], ps2[:], AF.Relu, bias=b2[:])
            ps3 = small_ps.tile([1, n], f32, tag="mlp")
            nc.tensor.matmul(ps3[:], lhsT=w3[:], rhs=h2[:], start=True, stop=True)
            nc.scalar.activation(imp_dst, ps3[:], AF.Exp, bias=nb3[:], scale=-1.0)

        def sigma_inplace(ap):
            nc.vector.tensor_scalar_add(ap, ap, 1.0)
            nc.vector.reciprocal(ap, ap)

        def interact_grp(g):
            """heads hh=4g..4g+3: interaction + combine -> estage3/estage4."""
            # qk = qimp (x) kimp outer product, off the critical path
            qg = sb.tile([4, NBH], f32, tag="qg")
            nc.gpsimd.dma_start(
                qg[:], qimp[0:1, g * 128:(g + 1) * 128]
                .rearrange("o (c i) -> o c i", c=4))
            kg = sb.tile([4, NB], f32, tag="kg")
            nc.gpsimd.dma_start(
                kg[:], kimp[0:1, g * 256:(g + 1) * 256]
                .rearrange("o (c j) -> o c j", c=4))
            qk = sb.tile([4, NBH * NB], f32, tag="qk")
            nc.vector.tensor_tensor(
                qk[:].rearrange("p (i j) -> p i j", i=NBH),
                qg[:].unsqueeze(2).broadcast_to((4, NBH, NB)),
                kg[:].unsqueeze(1).broadcast_to((4, NBH, NB)),
                op=OP.mult)

            psq = tp_ps.tile([128, NBH], f32, tag="tp")
            for cc in range(4):
                nc.tensor.matmul(psq[32 * cc:32 * cc + 32, :], lhsT=w1a[:],
                                 rhs=qTall[:, (4 * g + cc) * NBH:(4 * g + cc + 1) * NBH],
                                 tile_position=(0, 32 * cc), start=True, stop=True)
            qp4 = sb.tile([128, NBH], f32, tag="qp4")
            nc.scalar.copy(qp4[:], psq[:])
            psk = tp_ps.tile([128, NB], f32, tag="tp")
            for cc in range(4):
                nc.tensor.matmul(psk[32 * cc:32 * cc + 32, :], lhsT=w1b[:],
                                 rhs=kTall[:, (4 * g + cc) * NB:(4 * g + cc + 1) * NB],
                                 tile_position=(0, 32 * cc), start=True, stop=True)
            kp4 = sb.tile([128, NB], f32, tag="kp4")
            nc.scalar.activation(kp4[:], psk[:], AF.Identity, bias=b1i[:])
            hh = hpool.tile([128, NBH * NB], f32, tag="hh")
            nc.vector.tensor_tensor(
                hh[:].rearrange("p (i j) -> p i j", i=NBH),
                qp4[:].unsqueeze(2).broadcast_to((128, NBH, NB)),
                kp4[:].unsqueeze(1).broadcast_to((128, NBH, NB)),
                op=OP.add)
            nc.scalar.activation(hh[:], hh[:], AF.Relu)
            e4 = sb.tile([4, NBH * NB], f32, tag="e4")
            for n in range(4):
                sl = slice(n * 512, (n + 1) * 512)
                psI = int_ps.tile([4, 512], f32, tag="int")
                nc.tensor.matmul(psI[:], lhsT=w2bd[:], rhs=hh[:, sl],
                                 start=True, stop=True)
                nc.scalar.activation(e4[:, sl], psI[:], AF.Exp,
                                     bias=nb2i[:], scale=-1.0)
                nc.scalar.activation(e4[:, sl], e4[:, sl], AF.Identity,
                                     bias=one4[:])
                nc.vector.reciprocal(e4[:, sl], e4[:, sl])
                nc.vector.tensor_tensor(e4[:, sl], e4[:, sl], qk[:, sl],
                                        op=OP.mult)
            # scatter head rows into estage3 [(32 i) p, 64 j @ hh*64]
            for cc in range(4):
                nc.scalar.dma_start(
                    estage3[:, (4 * g + cc) * NB:(4 * g + cc + 1) * NB],
                    e4[cc:cc + 1, :])
            # incremental (hh,j)->(j,hh) transpose for this grp's columns
            nc.scalar.copy(
                estage4[:].rearrange("p (j hh) -> p hh j", j=NB, hh=16)
                [:, 4 * g:4 * g + 4, :],
                estage3[:, 4 * g * NB:(4 * g + 4) * NB]
                .rearrange("p (hh j) -> p hh j", hh=4, j=NB))

        # ---- emit program ----
        for h in range(H):
            pool_dve(ks, h, bigk, kPack, "k")
        nc.gpsimd.dma_start(cc_in[:], kPack[:])
        nc.gpsimd.collective_compute(
            "AllGather", mybir.AluOpType.bypass,
            replica_groups=[[0, 1], [2, 3], [4, 5], [6, 7]],
            ins=[cc_in.opt()], outs=[cc_out.opt()])
        nc.gpsimd.dma_start(ktmp[:, 0:H * NBH], cc_out[0:128])
        nc.gpsimd.dma_start(ktmp[:, H * NBH:], cc_out[128:256])
        # interleave on ACT: kTall[(h, half, j)] <- ktmp[(half, h, j)]
        nc.scalar.copy(
            kTall[:].rearrange("p (h c j) -> p c h j", h=H, c=2, j=NBH),
            ktmp[:].rearrange("p (c h j) -> p c h j", c=2, h=H, j=NBH))

        def q_grp(g):
            paths = [pool_dve, pool_pe, pool_pe, pool_dve] if g in (0, 3) \
                else [pool_pe, pool_dve, pool_pe, pool_pe]
            for cc in range(4):
                paths[cc](qs, 4 * g + cc, bigq, qTall, "q")

        def grp_compute(g):
            mlp(qTall, g * 128, (g + 1) * 128, qimp[0:1, g * 128:(g + 1) * 128])
            mlp(kTall, g * 256, (g + 1) * 256, kimp[0:1, g * 256:(g + 1) * 256])
            sigma_inplace(qimp[0:1, g * 128:(g + 1) * 128])
            sigma_inplace(kimp[0:1, g * 256:(g + 1) * 256])
            interact_grp(g)

        q_grp(0)
        q_grp(1)
        grp_compute(0)
        q_grp(2)
        grp_compute(1)
        grp_compute(2)
        q_grp(3)
        grp_compute(3)

        # fold into foldedT [p=(r,a,jq), (jr,hh)], j = jq*16 + jr
        for rr in range(NROW):
            eng = nc.sync if rr % 2 == 0 else nc.scalar
            eng.dma_start(
                foldedT[16 * rr:16 * rr + 16, :]
                .rearrange("p (jr hh) -> p jr hh", jr=16, hh=16),
                estage4[4 * rr:4 * rr + 4, :]
                .rearrange("p (jq jr hh) -> p jq jr hh", jq=4, jr=16, hh=16))

        # ---- top-k: two-phase exact fp32 bisection (transposed layout) ----
        thr = persist.tile([128, 1], f32, tag="thr")
        pr = persist.tile([128, 1], f32, tag="pr")
        lo = persist.tile([128, 1], f32, tag="lo")
        ge = persist.tile([128, 256], f32, tag="ge")
        cntp = persist.tile([128, 1], f32, tag="cntp")

        def bisect(vals, nit):
            nc.vector.memset(thr[:], 0.5)
            s = 0.25
            for _ in range(nit):
                psC = small_ps.tile([128, 1], f32, tag="mlp")
                nc.vector.tensor_tensor(
                    ge[:], vals[:], thr[:].broadcast_to((128, 256)),
                    op=OP.is_ge)
                nc.vector.tensor_reduce(cntp[:], ge[:], axis=AX.X, op=OP.add)
                nc.tensor.matmul(psC[:], lhsT=blk16[:], rhs=cntp[:],
                                 start=True, stop=True)
                # pr = (cnt >= K) * 2s ; thr += pr - s
                nc.vector.tensor_scalar(pr[:], psC[:], float(KSEL), 2.0 * s,
                                        op0=OP.is_ge, op1=OP.mult)
                nc.vector.scalar_tensor_tensor(thr[:], pr[:], -s, thr[:],
                                               op0=OP.add, op1=OP.add)
                s *= 0.5
            return s

        s1 = bisect(foldedT, NIT1)
        nc.vector.tensor_scalar_add(lo[:], thr[:], -2.0 * s1)
        nc.vector.tensor_scalar(foldedT2[:], foldedT[:], lo[:],
                                float(2 ** NIT1), op0=OP.subtract, op1=OP.mult)
        s2 = bisect(foldedT2, NIT2)
        nc.vector.tensor_scalar_add(lo[:], thr[:], -2.0 * s2)

        mask = persist.tile([128, 256], u8, tag="mask")
        nc.vector.tensor_scalar(mask[:], foldedT2[:], lo[:], None, op0=OP.is_ge)
        nc.sync.dma_start(
            y[:].rearrange("r (a jq jr4) (jrm hh) -> (r a jq) jr4 jrm hh",
                           a=4, jq=4, jr4=4, jrm=4, hh=16),
            mask[:].rearrange("p (jr4 jrm hh) -> p jr4 jrm hh",
                              jr4=4, jrm=4, hh=16))

    nc.compile()
    return nc


def _constants(w_imp1, b_imp1, w_imp2, b_imp2, w_imp3, b_imp3,
               w_int1, b_int1, w_int2, b_int2):
    f = np.float32
    blkS = np.zeros((128, NBH), f)
    blkS[np.arange(128), np.arange(128) // 4] = 1.0 / 128.0
    blkq = np.zeros((128, 32), f)
    blkq[np.arange(128), np.arange(128) // 4] = 1.0
    blk16 = np.zeros((128, 128), f)
    blk16[np.arange(128)[:, None] // 16 == np.arange(128)[None, :] // 16] = 1.0
    consts = {
        "c_blkS": blkS,
        "c_blkq": blkq,
        "c_id32": (np.eye(32, dtype=f) / 128.0).astype(f),
        "c_blk16": blk16,
        "c_w1": np.ascontiguousarray(w_imp1, f),
        "c_b1": np.ascontiguousarray(np.asarray(b_imp1, f).reshape(HID1, 1)),
        "c_w2": np.ascontiguousarray(w_imp2, f),
        "c_b2": np.ascontiguousarray(np.asarray(b_imp2, f).reshape(HID2, 1)),
        "c_w3": np.ascontiguousarray(w_imp3, f),
        "c_nb3": np.ascontiguousarray(-np.asarray(b_imp3, f).reshape(1, 1)),
        "c_w1a": np.ascontiguousarray(np.asarray(w_int1, f)[:D]),
        "c_w1b": np.ascontiguousarray(np.asarray(w_int1, f)[D:]),
        "c_b1i": np.ascontiguousarray(
            np.tile(np.asarray(b_int1, f).reshape(HID1, 1), (4, 1))),
        "c_nb2i": np.ascontiguousarray(
            np.tile(-np.asarray(b_int2, f).reshape(1, 1), (4, 1))),
        "c_one4": np.ones((4, 1), f),
    }
    w2bd = np.zeros((128, 4), f)
    for c in range(4):
        w2bd[32 * c:32 * c + 32, c] = np.asarray(w_int2, f)[:, 0]
    consts["c_w2bd"] = w2bd
    return consts


def _in_maps(q, k, w_imp1, b_imp1, w_imp2, b_imp2, w_imp3, b_imp3,
             w_int1, b_int1, w_int2, b_int2):
    q = np.asarray(q, np.float32)
    k = np.asarray(k, np.float32)
    consts = _constants(w_imp1, b_imp1, w_imp2, b_imp2, w_imp3, b_imp3,
                        w_int1, b_int1, w_int2, b_int2)
    in_maps = []
    for c in range(NCORES):
        b, p = c // 2, c % 2
        m = {"qs": q[b, p * SH:(p + 1) * SH],
             "ks": k[b, p * SH:(p + 1) * SH]}
        m.update(consts)
        in_maps.append(m)
    return in_maps


def kernel(q, k, **w):
    from concourse.bass_utils import run_bass_kernel_spmd

    in_maps = _in_maps(q, k, **w)

    if "nc" not in _nc_cache:
        _nc_cache["nc"] = _build_nc()
    res = run_bass_kernel_spmd(_nc_cache["nc"], in_maps,
                               core_ids=list(range(NCORES)))
    out = np.empty((B, H, NB, NB), np.uint8)
    for c in range(NCORES):
        b, p = c // 2, c % 2
        out[b, p * 8:(p + 1) * 8] = res.results[c]["y"]
    return out > 0


# revision 38
# speedup vs baseline: 1.7301x; 1.0787x over previous
"""Trainium2 Bass kernel for ContentAdaptiveSparsity (topk_masking).

Reference semantics: combined[b,i,j,h] = q_imp[b,i,h] * k_imp[b,j,h] *
interaction[b,i,j,h] from block-mean pooled q,k (64 blocks of 128) through
tiny MLPs; raw row-major reshape [B,nb,nb,H] -> [B,16,4096]; top-1024 mask
per (b, r) row, candidate m = (i%4)*1024 + j*16 + h, out y[r, m//64, m%64].

Sharding: core c = (batch b=c//2, seq half p=c%2) reads the CONTIGUOUS
slices q[b, p*4096:...] / k[b, p*4096:...] (64 MB, zero-copy host views).
Its 32 i-blocks are exactly its 8 output rows r in [8p, 8p+8); its 32
j-blocks are half of k.  Core pairs exchange pooled-k halves (256 KB) via
pairwise AllGather through DRAM.

Device pipeline:
  - 32 per-head tile loads [128 chunks x (32 rows, 128 d)] on SP, gather
    APs straight off the raw [S,H,D] layout (512B descriptors).
  - pooling split across engines: k heads + first q heads on DVE (grouped
    free-dim reduce -> [128a,128d] partial, then one matmul lhsT=partial,
    rhs=chunk->block/128 that fuses cross-chunk sum + mean + transpose);
    remaining q heads on PE (block-diag ones matmuls accumulating
    [32,512] psum + small DVE grouped reduce + id32/128 transpose).
  - k first -> kPack -> DRAM -> AllGather(pair) -> kTall [128,(h,64)];
    per 4-head grp: importance MLPs, qk=qimp x kimp outer (off critical
    path), interaction grid add (DVE), relu (ACT), block-diag w2 matmul
    per 512-col slice pipelined with exp (ACT), +1 (ACT bias), recip
    (DVE), * qk (DVE); scatter rows to estage3, per-grp transpose copy
    (hh,j)->(j,hh) into estage4 (ACT).
  - fold: 8 DMAs estage4 -> foldedT [128=(r,a,jq), (jr,hh)] (1KB descs):
    row r occupies partitions 16r..16r+16; its 4096 candidates are
    (a,jq | jr,hh) with j = jq*16+jr.
  - top-k: two-phase exact fp32 bisection from (0,1], compile-time
    power-of-2 steps (thr sums EXACT in fp32).  Per iter: DVE compare +
    per-partition count, blk16 block-diag matmul summing each row's 16
    partitions (result replicated across the group), then two fused
    threshold-update ops (tensor_scalar is_ge*2s, scalar_tensor_tensor).
    After 17 iters bracket width 2^-17; rebase foldedT2 =
    (foldedT - L)*2^17 in ONE fused op (Sterbenz-exact near the bracket),
    11 more iters -> resolution 2^-28, well under the min candidate gap
    (~1.5e-8).  Mask = (foldedT2 >= L2) via per-partition scalar compare.
"""

import numpy as np

B, S, H, D = 4, 8192, 16, 128
SH = S // 2       # per-core seq slice
NB = 64           # blocks per sequence
NBH = 32          # blocks per core half
NROW = 8          # topk rows (r) per core
NCORES = 8
KSEL = 1024
HID1, HID2 = 32, 16
NIT1, NIT2 = 17, 10
N_Q_DVE = 4       # q heads pooled on the DVE path (rest on PE)

_nc_cache = {}


def _build_nc():
    from contextlib import ExitStack

    from concourse import bacc
    import concourse.mybir as mybir
    from concourse.tile import TileContext

    f32 = mybir.dt.float32
    u8 = mybir.dt.uint8
    AF = mybir.ActivationFunctionType
    OP = mybir.AluOpType
    AX = mybir.AxisListType

    nc = bacc.Bacc("TRN2", target_bir_lowering=False, debug=False,
                   num_devices=NCORES)

    qs = nc.dram_tensor("qs", [SH, H, D], f32, kind="ExternalInput")
    ks = nc.dram_tensor("ks", [SH, H, D], f32, kind="ExternalInput")
    c_blkS = nc.dram_tensor("c_blkS", [128, NBH], f32, kind="ExternalInput")
    c_blkq = nc.dram_tensor("c_blkq", [128, 32], f32, kind="ExternalInput")
    c_id32 = nc.dram_tensor("c_id32", [32, 32], f32, kind="ExternalInput")
    c_blk16 = nc.dram_tensor("c_blk16", [128, 128], f32, kind="ExternalInput")
    c_w1 = nc.dram_tensor("c_w1", [D, HID1], f32, kind="ExternalInput")
    c_b1 = nc.dram_tensor("c_b1", [HID1, 1], f32, kind="ExternalInput")
    c_w2 = nc.dram_tensor("c_w2", [HID1, HID2], f32, kind="ExternalInput")
    c_b2 = nc.dram_tensor("c_b2", [HID2, 1], f32, kind="ExternalInput")
    c_w3 = nc.dram_tensor("c_w3", [HID2, 1], f32, kind="ExternalInput")
    c_nb3 = nc.dram_tensor("c_nb3", [1, 1], f32, kind="ExternalInput")
    c_w1a = nc.dram_tensor("c_w1a", [D, HID1], f32, kind="ExternalInput")
    c_w1b = nc.dram_tensor("c_w1b", [D, HID1], f32, kind="ExternalInput")
    c_b1i = nc.dram_tensor("c_b1i", [128, 1], f32, kind="ExternalInput")
    c_w2bd = nc.dram_tensor("c_w2bd", [128, 4], f32, kind="ExternalInput")
    c_nb2i = nc.dram_tensor("c_nb2i", [4, 1], f32, kind="ExternalInput")
    c_one4 = nc.dram_tensor("c_one4", [4, 1], f32, kind="ExternalInput")

    y = nc.dram_tensor("y", [NROW, NB, NB], u8, kind="ExternalOutput")

    with TileContext(nc) as tc, ExitStack() as ctx:
        const = ctx.enter_context(tc.tile_pool(name="const", bufs=1))
        bigk = ctx.enter_context(tc.tile_pool(name="bigk", bufs=3))
        bigq = ctx.enter_context(tc.tile_pool(name="bigq", bufs=4))
        part_pool = ctx.enter_context(tc.tile_pool(name="part", bufs=3))
        hpool = ctx.enter_context(tc.tile_pool(name="hpool", bufs=2))
        sb = ctx.enter_context(tc.tile_pool(name="sb", bufs=2))
        persist = ctx.enter_context(tc.tile_pool(name="persist", bufs=1))
        dram = ctx.enter_context(tc.tile_pool(name="dram", bufs=1, space="DRAM"))
        pool_ps = ctx.enter_context(tc.tile_pool(name="pool_ps", bufs=1, space="PSUM"))
        pool2_ps = ctx.enter_context(tc.tile_pool(name="pool2_ps", bufs=2, space="PSUM"))
        tp_ps = ctx.enter_context(tc.tile_pool(name="tp_ps", bufs=2, space="PSUM"))
        small_ps = ctx.enter_context(tc.tile_pool(name="small_ps", bufs=1, space="PSUM"))
        int_ps = ctx.enter_context(tc.tile_pool(name="int_ps", bufs=2, space="PSUM"))

        def cload(dramt, shape, tag):
            t = const.tile(shape, f32, tag=tag)
            nc.gpsimd.dma_start(t[:], dramt[:])
            return t

        blkS = cload(c_blkS, [128, NBH], "blkS")
        blkq = cload(c_blkq, [128, 32], "blkq")
        id32 = cload(c_id32, [32, 32], "id32")
        blk16 = cload(c_blk16, [128, 128], "blk16")
        w1 = cload(c_w1, [D, HID1], "w1")
        b1 = cload(c_b1, [HID1, 1], "b1")
        w2 = cload(c_w2, [HID1, HID2], "w2")
        b2 = cload(c_b2, [HID2, 1], "b2")
        w3 = cload(c_w3, [HID2, 1], "w3")
        nb3 = cload(c_nb3, [1, 1], "nb3")
        w1a = cload(c_w1a, [D, HID1], "w1a")
        w1b = cload(c_w1b, [D, HID1], "w1b")
        b1i = cload(c_b1i, [128, 1], "b1i")
        w2bd = cload(c_w2bd, [128, 4], "w2bd")
        nb2i = cload(c_nb2i, [4, 1], "nb2i")
        one4 = cload(c_one4, [4, 1], "one4")

        qTall = persist.tile([128, H * NBH], f32, tag="qTall")    # (h, i32)
        kPack = persist.tile([128, H * NBH], f32, tag="kPack")    # (h, j32) mine
        kTall = persist.tile([128, H * NB], f32, tag="kTall")     # (h, j64)
        ktmp = persist.tile([128, H * NB], f32, tag="ktmp")       # (half, h, j32)
        qimp = persist.tile([1, H * NBH], f32, tag="qimp")
        kimp = persist.tile([1, H * NB], f32, tag="kimp")
        estage3 = persist.tile([NBH, H * NB], f32, tag="estage3")  # [i, (hh,j)]
        estage4 = persist.tile([NBH, H * NB], f32, tag="estage4")  # [i, (j,hh)]
        foldedT = persist.tile([128, 256], f32, tag="foldedT")
        foldedT2 = persist.tile([128, 256], f32, tag="foldedT2")

        cc_in = dram.tile([128, H * NBH], f32)
        cc_out = dram.tile([2 * 128, H * NBH], f32)

        def pool_dve(src, h, big, dst, dtag):
            """DVE-path block-mean pool of head h -> dst cols [h*32,(h+1)*32)."""
            xt = big.tile([128, 4096], f32, tag=dtag)
            nc.sync.dma_start(
                xt[:].rearrange("a (b d) -> a b d", b=32),
                src[:, h, :].rearrange("(a b) d -> a b d", a=128))
            part = part_pool.tile([128, 128], f32, tag="part")
            nc.vector.tensor_reduce(
                part[:], xt[:].rearrange("a (b d) -> a d b", b=32),
                axis=AX.X, op=OP.add)
            ps = pool_ps.tile([128, NBH], f32, tag="pool")
            nc.tensor.matmul(ps[:], lhsT=part[:], rhs=blkS[:],
                             start=True, stop=True)
            nc.scalar.copy(dst[:, h * NBH:(h + 1) * NBH], ps[:])

        def pool_pe(src, h, big, dst, dtag):
            """PE-path block-mean pool of head h -> dst cols [h*32,(h+1)*32)."""
            xt = big.tile([128, 4096], f32, tag=dtag)
            nc.sync.dma_start(
                xt[:].rearrange("a (b d) -> a b d", b=32),
                src[:, h, :].rearrange("(a b) d -> a b d", a=128))
            ps = pool2_ps.tile([32, 512], f32, tag="pool2")
            for j in range(8):
                nc.tensor.matmul(ps[:], lhsT=blkq[:], rhs=xt[:, j * 512:(j + 1) * 512],
                                 start=(j == 0), stop=(j == 7))
            sbp = sb.tile([32, 128], f32, tag="sbp")
            nc.vector.tensor_reduce(
                sbp[:], ps[:].rearrange("m (s d) -> m d s", s=4, d=128),
                axis=AX.X, op=OP.add)
            pst = tp_ps.tile([128, NBH], f32, tag="tp")
            nc.tensor.matmul(pst[:], lhsT=sbp[:], rhs=id32[:], start=True, stop=True)
            nc.scalar.copy(dst[:, h * NBH:(h + 1) * NBH], pst[:])

        def mlp(xT, n0, n1, imp_dst):
            """importance MLP on xT cols [n0:n1] -> imp_dst (exp(-logit))."""
            n = n1 - n0
            ps1 = small_ps.tile([HID1, n], f32, tag="mlp")
            nc.tensor.matmul(ps1[:], lhsT=w1[:], rhs=xT[:, n0:n1], start=True, stop=True)
            h1 = sb.tile([HID1, n], f32, tag="h1")
            nc.scalar.activation(h1[:], ps1[:], AF.Relu, bias=b1[:])
            ps2 = small_ps.tile([HID2, n], f32, tag="mlp")
            nc.tensor.matmul(ps2[:], lhsT=w2[:], rhs=h1[:], start=True, stop=True)
            h2 = sb.tile([HID2, n], f32, tag="h2")
            nc.scalar.activation(h2[:], ps2[:], AF.Relu, bias=b2[:])
            ps3 = small_ps.tile([1, n], f32, tag="mlp")
            nc.tensor.matmul(ps3[:], lhsT=w3[:], rhs=h2[:], start=True, stop=True)
            nc.scalar.activation(imp_dst, ps3[:], AF.Exp, bias=nb3[:], scale=-1.0)

        def sigma_inplace(ap):
            nc.vector.tensor_scalar_add(ap, ap, 1.0)
            nc.vector.reciprocal(ap, ap)

        def interact_grp(g):
            """heads hh=4g..4g+3: interaction + combine -> estage3/estage4."""
            # qk = qimp (x) kimp outer product, off the critical path
            qg = sb.tile([4, NBH], f32, tag="qg")
            nc.gpsimd.dma_start(
                qg[:], qimp[0:1, g * 128:(g + 1) * 128]
                .rearrange("o (c i) -> o c i", c=4))
            kg = sb.tile([4, NB], f32, tag="kg")
            nc.gpsimd.dma_start(
                kg[:], kimp[0:1, g * 256:(g + 1) * 256]
                .rearrange("o (c j) -> o c j", c=4))
            qk = sb.tile([4, NBH * NB], f32, tag="qk")
            nc.vector.tensor_tensor(
                qk[:].rearrange("p (i j) -> p i j", i=NBH),
                qg[:].unsqueeze(2).broadcast_to((4, NBH, NB)),
                kg[:].unsqueeze(1).broadcast_to((4, NBH, NB)),
                op=OP.mult)

            psq = tp_ps.tile([128, NBH], f32, tag="tp")
            for cc in range(4):
                nc.tensor.matmul(psq[32 * cc:32 * cc + 32, :], lhsT=w1a[:],
                                 rhs=qTall[:, (4 * g + cc) * NBH:(4 * g + cc + 1) * NBH],
                                 tile_position=(0, 32 * cc), start=True, stop=True)
            qp4 = sb.tile([128, NBH], f32, tag="qp4")
            nc.scalar.copy(qp4[:], psq[:])
            psk = tp_ps.tile([128, NB], f32, tag="tp")
            for cc in range(4):
                nc.tensor.matmul(psk[32 * cc:32 * cc + 32, :], lhsT=w1b[:],
                                 rhs=kTall[:, (4 * g + cc) * NB:(4 * g + cc + 1) * NB],
                                 tile_position=(0, 32 * cc), start=True, stop=True)
            kp4 = sb.tile([128, NB], f32, tag="kp4")
            nc.scalar.activation(kp4[:], psk[:], AF.Identity, bias=b1i[:])
            hh = hpool.tile([128, NBH * NB], f32, tag="hh")
            nc.vector.tensor_tensor(
                hh[:].rearrange("p (i j) -> p i j", i=NBH),
                qp4[:].unsqueeze(2).broadcast_to((128, NBH, NB)),
                kp4[:].unsqueeze(1).broadcast_to((128, NBH, NB)),
                op=OP.add)
            nc.scalar.activation(hh[:], hh[:], AF.Relu)
            e4 = sb.tile([4, NBH * NB], f32, tag="e4")
            for n in range(4):
                sl = slice(n * 512, (n + 1) * 512)
                psI = int_ps.tile([4, 512], f32, tag="int")
                nc.tensor.matmul(psI[:], lhsT=w2bd[:], rhs=hh[:, sl],
                                 start=True, stop=True)
                nc.scalar.activation(e4[:, sl], psI[:], AF.Exp,
                                     bias=nb2i[:], scale=-1.0)
                nc.scalar.activation(e4[:, sl], e4[:, sl], AF.Identity,
                                     bias=one4[:])
                nc.vector.reciprocal(e4[:, sl], e4[:, sl])
                nc.vector.tensor_tensor(e4[:, sl], e4[:, sl], qk[:, sl],
                                        op=OP.mult)
            # scatter head rows into estage3 [(32 i) p, 64 j @ hh*64]
            for cc in range(4):
                nc.scalar.dma_start(
                    estage3[:, (4 * g + cc) * NB:(4 * g + cc + 1) * NB],
                    e4[cc:cc + 1, :])
            # incremental (hh,j)->(j,hh) transpose for this grp's columns
            nc.scalar.copy(
                estage4[:].rearrange("p (j hh) -> p hh j", j=NB, hh=16)
                [:, 4 * g:4 * g + 4, :],
                estage3[:, 4 * g * NB:(4 * g + 4) * NB]
                .rearrange("p (hh j) -> p hh j", hh=4, j=NB))

        # ---- emit program ----
        for h in range(H):
            pool_dve(ks, h, bigk, kPack, "k")
        nc.sync.dma_start(cc_in[:], kPack[:])
        nc.gpsimd.collective_compute(
            "AllGather", mybir.AluOpType.bypass,
            replica_groups=[[0, 1], [2, 3], [4, 5], [6, 7]],
            ins=[cc_in.opt()], outs=[cc_out.opt()])
        def q_grp(g):
            paths = [pool_dve, pool_pe, pool_pe, pool_dve] if g in (0, 3) \
                else [pool_pe, pool_dve, pool_pe, pool_pe]
            for cc in range(4):
                paths[cc](qs, 4 * g + cc, bigq, qTall, "q")

        def grp_compute(g):
            mlp(qTall, g * 128, (g + 1) * 128, qimp[0:1, g * 128:(g + 1) * 128])
            mlp(kTall, g * 256, (g + 1) * 256, kimp[0:1, g * 256:(g + 1) * 256])
            sigma_inplace(qimp[0:1, g * 128:(g + 1) * 128])
            sigma_inplace(kimp[0:1, g * 256:(g + 1) * 256])
            interact_grp(g)

        q_grp(0)
        q_grp(1)
        nc.sync.dma_start(ktmp[:, 0:H * NBH], cc_out[0:128])
        nc.sync.dma_start(ktmp[:, H * NBH:], cc_out[128:256])
        # interleave on Pool (idle engine; ACT would head-block pool copies)
        nc.gpsimd.tensor_copy(
            kTall[:].rearrange("p (h c j) -> p c h j", h=H, c=2, j=NBH),
            ktmp[:].rearrange("p (c h j) -> p c h j", c=2, h=H, j=NBH))
        grp_compute(0)
        q_grp(2)
        grp_compute(1)
        grp_compute(2)
        q_grp(3)
        grp_compute(3)

        # fold into foldedT [p=(r,a,jq), (jr,hh)], j = jq*16 + jr
        for rr in range(NROW):
            eng = nc.sync if rr % 2 == 0 else nc.scalar
            eng.dma_start(
                foldedT[16 * rr:16 * rr + 16, :]
                .rearrange("p (jr hh) -> p jr hh", jr=16, hh=16),
                estage4[4 * rr:4 * rr + 4, :]
                .rearrange("p (jq jr hh) -> p jq jr hh", jq=4, jr=16, hh=16))

        # ---- top-k: two-phase exact fp32 bisection (transposed layout) ----
        thr = persist.tile([128, 1], f32, tag="thr")
        pr = persist.tile([128, 1], f32, tag="pr")
        lo = persist.tile([128, 1], f32, tag="lo")
        ge = persist.tile([128, 256], f32, tag="ge")
        cntp = persist.tile([128, 1], f32, tag="cntp")

        def bisect(vals, nit):
            nc.vector.memset(thr[:], 0.5)
            s = 0.25
            for _ in range(nit):
                psC = small_ps.tile([128, 1], f32, tag="mlp")
                nc.vector.tensor_tensor(
                    ge[:], vals[:], thr[:].broadcast_to((128, 256)),
                    op=OP.is_ge)
                nc.vector.tensor_reduce(cntp[:], ge[:], axis=AX.X, op=OP.add)
                nc.tensor.matmul(psC[:], lhsT=blk16[:], rhs=cntp[:],
                                 start=True, stop=True)
                # pr = (cnt >= K) * 2s ; thr += pr - s
                nc.vector.tensor_scalar(pr[:], psC[:], float(KSEL), 2.0 * s,
                                        op0=OP.is_ge, op1=OP.mult)
                nc.vector.scalar_tensor_tensor(thr[:], pr[:], -s, thr[:],
                                               op0=OP.add, op1=OP.add)
                s *= 0.5
            return s

        s1 = bisect(foldedT, NIT1)
        nc.vector.tensor_scalar_add(lo[:], thr[:], -2.0 * s1)
        nc.vector.tensor_scalar(foldedT2[:], foldedT[:], lo[:],
                                float(2 ** NIT1), op0=OP.subtract, op1=OP.mult)
        s2 = bisect(foldedT2, NIT2)
        nc.vector.tensor_scalar_add(lo[:], thr[:], -2.0 * s2)

        mask = persist.tile([128, 256], u8, tag="mask")
        nc.vector.tensor_scalar(mask[:], foldedT2[:], lo[:], None, op0=OP.is_ge)
        nc.sync.dma_start(
            y[:].rearrange("r (a jq jr4) (jrm hh) -> (r a jq) jr4 jrm hh",
                           a=4, jq=4, jr4=4, jrm=4, hh=16),
            mask[:].rearrange("p (jr4 jrm hh) -> p jr4 jrm hh",
                              jr4=4, jrm=4, hh=16))

    nc.compile()
    return nc


def _constants(w_imp1, b_imp1, w_imp2, b_imp2, w_imp3, b_imp3,
               w_int1, b_int1, w_int2, b_int2):
    f = np.float32
    blkS = np.zeros((128, NBH), f)
    blkS[np.arange(128), np.arange(128) // 4] = 1.0 / 128.0
    blkq = np.zeros((128, 32), f)
    blkq[np.arange(128), np.arange(128) // 4] = 1.0
    blk16 = np.zeros((128, 128), f)
    blk16[np.arange(128)[:, None] // 16 == np.arange(128)[None, :] // 16] = 1.0
    consts = {
        "c_blkS": blkS,
        "c_blkq": blkq,
        "c_id32": (np.eye(32, dtype=f) / 128.0).astype(f),
        "c_blk16": blk16,
        "c_w1": np.ascontiguousarray(w_imp1, f),
        "c_b1": np.ascontiguousarray(np.asarray(b_imp1, f).reshape(HID1, 1)),
        "c_w2": np.ascontiguousarray(w_imp2, f),
        "c_b2": np.ascontiguousarray(np.asarray(b_imp2, f).reshape(HID2, 1)),
        "c_w3": np.ascontiguousarray(w_imp3, f),
        "c_nb3": np.ascontiguousarray(-np.asarray(b_imp3, f).reshape(1, 1)),
        "c_w1a": np.ascontiguousarray(np.asarray(w_int1, f)[:D]),
        "c_w1b": np.ascontiguousarray(np.asarray(w_int1, f)[D:]),
        "c_b1i": np.ascontiguousarray(
            np.tile(np.asarray(b_int1, f).reshape(HID1, 1), (4, 1))),
        "c_nb2i": np.ascontiguousarray(
            np.tile(-np.asarray(b_int2, f).reshape(1, 1), (4, 1))),
        "c_one4": np.ones((4, 1), f),
    }
    w2bd = np.zeros((128, 4), f)
    for c in range(4):
        w2bd[32 * c:32 * c + 32, c] = np.asarray(w_int2, f)[:, 0]
    consts["c_w2bd"] = w2bd
    return consts


def _in_maps(q, k, w_imp1, b_imp1, w_imp2, b_imp2, w_imp3, b_imp3,
             w_int1, b_int1, w_int2, b_int2):
    q = np.asarray(q, np.float32)
    k = np.asarray(k, np.float32)
    consts = _constants(w_imp1, b_imp1, w_imp2, b_imp2, w_imp3, b_imp3,
                        w_int1, b_int1, w_int2, b_int2)
    in_maps = []
    for c in range(NCORES):
        b, p = c // 2, c % 2
        m = {"qs": q[b, p * SH:(p + 1) * SH],
             "ks": k[b, p * SH:(p + 1) * SH]}
        m.update(consts)
        in_maps.append(m)
    return in_maps


def kernel(q, k, **w):
    from concourse.bass_utils import run_bass_kernel_spmd

    in_maps = _in_maps(q, k, **w)

    if "nc" not in _nc_cache:
        _nc_cache["nc"] = _build_nc()
    res = run_bass_kernel_spmd(_nc_cache["nc"], in_maps,
                               core_ids=list(range(NCORES)))
    out = np.empty((B, H, NB, NB), np.uint8)
    for c in range(NCORES):
        b, p = c // 2, c % 2
        out[b, p * 8:(p + 1) * 8] = res.results[c]["y"]
    return out > 0


# revision 45
# speedup vs baseline: 1.7545x; 1.0141x over previous
"""Trainium2 Bass kernel for ContentAdaptiveSparsity (topk_masking).

Reference semantics: combined[b,i,j,h] = q_imp[b,i,h] * k_imp[b,j,h] *
interaction[b,i,j,h] from block-mean pooled q,k (64 blocks of 128) through
tiny MLPs; raw row-major reshape [B,nb,nb,H] -> [B,16,4096]; top-1024 mask
per (b, r) row, candidate m = (i%4)*1024 + j*16 + h, out y[r, m//64, m%64].

Sharding: core c = (batch b=c//2, seq half p=c%2) reads the CONTIGUOUS
slices q[b, p*4096:...] / k[b, p*4096:...] (64 MB, zero-copy host views).
Its 32 i-blocks are exactly its 8 output rows r in [8p, 8p+8); its 32
j-blocks are half of k.  Core pairs exchange pooled-k halves (256 KB) via
pairwise AllGather through DRAM.

Device pipeline:
  - 32 per-head tile loads [128 chunks x (32 rows, 128 d)] on SP, gather
    APs straight off the raw [S,H,D] layout (512B descriptors).
  - pooling split across engines: k heads + first q heads on DVE (grouped
    free-dim reduce -> [128a,128d] partial, then one matmul lhsT=partial,
    rhs=chunk->block/128 that fuses cross-chunk sum + mean + transpose);
    remaining q heads on PE (block-diag ones matmuls accumulating
    [32,512] psum + small DVE grouped reduce + id32/128 transpose).
  - k first -> kPack -> DRAM -> AllGather(pair) -> kTall [128,(h,64)];
    per 4-head grp: importance MLPs, qk=qimp x kimp outer (off critical
    path), interaction grid add (DVE), relu (ACT), block-diag w2 matmul
    per 512-col slice pipelined with exp (ACT), +1 (ACT bias), recip
    (DVE), * qk (DVE); scatter rows to estage3, per-grp transpose copy
    (hh,j)->(j,hh) into estage4 (ACT).
  - fold: 8 DMAs estage4 -> foldedT [128=(r,a,jq), (jr,hh)] (1KB descs):
    row r occupies partitions 16r..16r+16; its 4096 candidates are
    (a,jq | jr,hh) with j = jq*16+jr.
  - top-k: two-phase exact fp32 bisection from (0,1], compile-time
    power-of-2 steps (thr sums EXACT in fp32).  Per iter: DVE compare +
    per-partition count, blk16 block-diag matmul summing each row's 16
    partitions (result replicated across the group), then two fused
    threshold-update ops (tensor_scalar is_ge*2s, scalar_tensor_tensor).
    After 17 iters bracket width 2^-17; rebase foldedT2 =
    (foldedT - L)*2^17 in ONE fused op (Sterbenz-exact near the bracket),
    11 more iters -> resolution 2^-28, well under the min candidate gap
    (~1.5e-8).  Mask = (foldedT2 >= L2) via per-partition scalar compare.
"""

import numpy as np

B, S, H, D = 4, 8192, 16, 128
SH = S // 2       # per-core seq slice
NB = 64           # blocks per sequence
NBH = 32          # blocks per core half
NROW = 8          # topk rows (r) per core
NCORES = 8
KSEL = 1024
HID1, HID2 = 32, 16
NIT1, NIT2 = 17, 10
N_Q_DVE = 4       # q heads pooled on the DVE path (rest on PE)

_nc_cache = {}


def _build_nc():
    from contextlib import ExitStack

    from concourse import bacc
    import concourse.mybir as mybir
    from concourse.tile import TileContext

    f32 = mybir.dt.float32
    u8 = mybir.dt.uint8
    AF = mybir.ActivationFunctionType
    OP = mybir.AluOpType
    AX = mybir.AxisListType

    nc = bacc.Bacc("TRN2", target_bir_lowering=False, debug=False,
                   num_devices=NCORES)

    qs = nc.dram_tensor("qs", [SH, H, D], f32, kind="ExternalInput")
    ks = nc.dram_tensor("ks", [SH, H, D], f32, kind="ExternalInput")
    c_blkS = nc.dram_tensor("c_blkS", [128, NBH], f32, kind="ExternalInput")
    c_blkq = nc.dram_tensor("c_blkq", [128, 32], f32, kind="ExternalInput")
    c_id32 = nc.dram_tensor("c_id32", [32, 32], f32, kind="ExternalInput")
    c_blk16 = nc.dram_tensor("c_blk16", [128, 128], f32, kind="ExternalInput")
    c_w1 = nc.dram_tensor("c_w1", [D, HID1], f32, kind="ExternalInput")
    c_b1 = nc.dram_tensor("c_b1", [HID1, 1], f32, kind="ExternalInput")
    c_w2 = nc.dram_tensor("c_w2", [HID1, HID2], f32, kind="ExternalInput")
    c_b2 = nc.dram_tensor("c_b2", [HID2, 1], f32, kind="ExternalInput")
    c_w3 = nc.dram_tensor("c_w3", [HID2, 1], f32, kind="ExternalInput")
    c_nb3 = nc.dram_tensor("c_nb3", [1, 1], f32, kind="ExternalInput")
    c_w1a = nc.dram_tensor("c_w1a", [D, HID1], f32, kind="ExternalInput")
    c_w1b = nc.dram_tensor("c_w1b", [D, HID1], f32, kind="ExternalInput")
    c_b1i = nc.dram_tensor("c_b1i", [128, 1], f32, kind="ExternalInput")
    c_w2bd = nc.dram_tensor("c_w2bd", [128, 4], f32, kind="ExternalInput")
    c_nb2i = nc.dram_tensor("c_nb2i", [4, 1], f32, kind="ExternalInput")
    c_one4 = nc.dram_tensor("c_one4", [4, 1], f32, kind="ExternalInput")

    y = nc.dram_tensor("y", [NROW, NB, NB], u8, kind="ExternalOutput")

    with TileContext(nc) as tc, ExitStack() as ctx:
        const = ctx.enter_context(tc.tile_pool(name="const", bufs=1))
        bigk = ctx.enter_context(tc.tile_pool(name="bigk", bufs=3))
        bigq = ctx.enter_context(tc.tile_pool(name="bigq", bufs=4))
        part_pool = ctx.enter_context(tc.tile_pool(name="part", bufs=3))
        hpool = ctx.enter_context(tc.tile_pool(name="hpool", bufs=2))
        sb = ctx.enter_context(tc.tile_pool(name="sb", bufs=2))
        persist = ctx.enter_context(tc.tile_pool(name="persist", bufs=1))
        dram = ctx.enter_context(tc.tile_pool(name="dram", bufs=1, space="DRAM"))
        pool_ps = ctx.enter_context(tc.tile_pool(name="pool_ps", bufs=1, space="PSUM"))
        pool2_ps = ctx.enter_context(tc.tile_pool(name="pool2_ps", bufs=2, space="PSUM"))
        tp_ps = ctx.enter_context(tc.tile_pool(name="tp_ps", bufs=2, space="PSUM"))
        small_ps = ctx.enter_context(tc.tile_pool(name="small_ps", bufs=1, space="PSUM"))
        int_ps = ctx.enter_context(tc.tile_pool(name="int_ps", bufs=2, space="PSUM"))

        def cload(dramt, shape, tag):
            t = const.tile(shape, f32, tag=tag)
            nc.gpsimd.dma_start(t[:], dramt[:])
            return t

        blkS = cload(c_blkS, [128, NBH], "blkS")
        blkq = cload(c_blkq, [128, 32], "blkq")
        id32 = cload(c_id32, [32, 32], "id32")
        blk16 = cload(c_blk16, [128, 128], "blk16")
        w1 = cload(c_w1, [D, HID1], "w1")
        b1 = cload(c_b1, [HID1, 1], "b1")
        w2 = cload(c_w2, [HID1, HID2], "w2")
        b2 = cload(c_b2, [HID2, 1], "b2")
        w3 = cload(c_w3, [HID2, 1], "w3")
        nb3 = cload(c_nb3, [1, 1], "nb3")
        w1a = cload(c_w1a, [D, HID1], "w1a")
        w1b = cload(c_w1b, [D, HID1], "w1b")
        b1i = cload(c_b1i, [128, 1], "b1i")
        w2bd = cload(c_w2bd, [128, 4], "w2bd")
        nb2i = cload(c_nb2i, [4, 1], "nb2i")
        one4 = cload(c_one4, [4, 1], "one4")

        qTall = persist.tile([128, H * NBH], f32, tag="qTall")    # (h, i32)
        kPack = persist.tile([128, H * NBH], f32, tag="kPack")    # (h, j32) mine
        kTall = persist.tile([128, H * NB], f32, tag="kTall")     # (h, j64)
        ktmp = persist.tile([128, H * NB], f32, tag="ktmp")       # (half, h, j32)
        qimp = persist.tile([1, H * NBH], f32, tag="qimp")
        kimp = persist.tile([1, H * NB], f32, tag="kimp")
        estage3 = persist.tile([NBH, H * NB], f32, tag="estage3")  # [i, (hh,j)]
        estage4 = persist.tile([NBH, H * NB], f32, tag="estage4")  # [i, (j,hh)]
        foldedT = persist.tile([128, 256], f32, tag="foldedT")
        foldedT2 = persist.tile([128, 256], f32, tag="foldedT2")

        cc_in = dram.tile([128, H * NBH], f32)
        cc_out = dram.tile([2 * 128, H * NBH], f32)

        def pool_dve(src, h, big, dst, dtag):
            """DVE-path block-mean pool of head h -> dst cols [h*32,(h+1)*32)."""
            xt = big.tile([128, 4096], f32, tag=dtag)
            nc.sync.dma_start(
                xt[:].rearrange("a (b d) -> a b d", b=32),
                src[:, h, :].rearrange("(a b) d -> a b d", a=128))
            part = part_pool.tile([128, 128], f32, tag="part")
            nc.vector.tensor_reduce(
                part[:], xt[:].rearrange("a (b d) -> a d b", b=32),
                axis=AX.X, op=OP.add)
            ps = pool_ps.tile([128, NBH], f32, tag="pool")
            nc.tensor.matmul(ps[:], lhsT=part[:], rhs=blkS[:],
                             start=True, stop=True)
            nc.scalar.copy(dst[:, h * NBH:(h + 1) * NBH], ps[:])

        def pool_pe(src, h, big, dst, dtag):
            """PE-path block-mean pool of head h -> dst cols [h*32,(h+1)*32)."""
            xt = big.tile([128, 4096], f32, tag=dtag)
            nc.sync.dma_start(
                xt[:].rearrange("a (b d) -> a b d", b=32),
                src[:, h, :].rearrange("(a b) d -> a b d", a=128))
            ps = pool2_ps.tile([32, 512], f32, tag="pool2")
            for j in range(8):
                nc.tensor.matmul(ps[:], lhsT=blkq[:], rhs=xt[:, j * 512:(j + 1) * 512],
                                 start=(j == 0), stop=(j == 7))
            sbp = sb.tile([32, 128], f32, tag="sbp")
            nc.vector.tensor_reduce(
                sbp[:], ps[:].rearrange("m (s d) -> m d s", s=4, d=128),
                axis=AX.X, op=OP.add)
            pst = tp_ps.tile([128, NBH], f32, tag="tp")
            nc.tensor.matmul(pst[:], lhsT=sbp[:], rhs=id32[:], start=True, stop=True)
            nc.scalar.copy(dst[:, h * NBH:(h + 1) * NBH], pst[:])

        def mlp(xT, n0, n1, imp_dst):
            """importance MLP on xT cols [n0:n1] -> imp_dst (exp(-logit))."""
            n = n1 - n0
            ps1 = small_ps.tile([HID1, n], f32, tag="mlp")
            nc.tensor.matmul(ps1[:], lhsT=w1[:], rhs=xT[:, n0:n1], start=True, stop=True)
            h1 = sb.tile([HID1, n], f32, tag="h1")
            nc.scalar.activation(h1[:], ps1[:], AF.Relu, bias=b1[:])
            ps2 = small_ps.tile([HID2, n], f32, tag="mlp")
            nc.tensor.matmul(ps2[:], lhsT=w2[:], rhs=h1[:], start=True, stop=True)
            h2 = sb.tile([HID2, n], f32, tag="h2")
            nc.scalar.activation(h2[:], ps2[:], AF.Relu, bias=b2[:])
            ps3 = small_ps.tile([1, n], f32, tag="mlp")
            nc.tensor.matmul(ps3[:], lhsT=w3[:], rhs=h2[:], start=True, stop=True)
            nc.scalar.activation(imp_dst, ps3[:], AF.Exp, bias=nb3[:], scale=-1.0)

        def sigma_inplace(ap):
            nc.vector.tensor_scalar_add(ap, ap, 1.0)
            nc.vector.reciprocal(ap, ap)

        def interact_grp(g):
            """heads hh=4g..4g+3: interaction + combine -> estage3/estage4."""
            # qk = qimp (x) kimp outer product, off the critical path
            qg = sb.tile([4, NBH], f32, tag="qg")
            nc.gpsimd.dma_start(
                qg[:], qimp[0:1, g * 128:(g + 1) * 128]
                .rearrange("o (c i) -> o c i", c=4))
            kg = sb.tile([4, NB], f32, tag="kg")
            nc.gpsimd.dma_start(
                kg[:], kimp[0:1, g * 256:(g + 1) * 256]
                .rearrange("o (c j) -> o c j", c=4))
            qk = sb.tile([4, NBH * NB], f32, tag="qk")
            nc.vector.tensor_tensor(
                qk[:].rearrange("p (i j) -> p i j", i=NBH),
                qg[:].unsqueeze(2).broadcast_to((4, NBH, NB)),
                kg[:].unsqueeze(1).broadcast_to((4, NBH, NB)),
                op=OP.mult)

            psq = tp_ps.tile([128, NBH], f32, tag="tp")
            for cc in range(4):
                nc.tensor.matmul(psq[32 * cc:32 * cc + 32, :], lhsT=w1a[:],
                                 rhs=qTall[:, (4 * g + cc) * NBH:(4 * g + cc + 1) * NBH],
                                 tile_position=(0, 32 * cc), start=True, stop=True)
            qp4 = sb.tile([128, NBH], f32, tag="qp4")
            nc.scalar.copy(qp4[:], psq[:])
            psk = tp_ps.tile([128, NB], f32, tag="tp")
            for cc in range(4):
                nc.tensor.matmul(psk[32 * cc:32 * cc + 32, :], lhsT=w1b[:],
                                 rhs=kTall[:, (4 * g + cc) * NB:(4 * g + cc + 1) * NB],
                                 tile_position=(0, 32 * cc), start=True, stop=True)
            kp4 = sb.tile([128, NB], f32, tag="kp4")
            nc.scalar.activation(kp4[:], psk[:], AF.Identity, bias=b1i[:])
            hh = hpool.tile([128, NBH * NB], f32, tag="hh")
            nc.vector.tensor_tensor(
                hh[:].rearrange("p (i j) -> p i j", i=NBH),
                qp4[:].unsqueeze(2).broadcast_to((128, NBH, NB)),
                kp4[:].unsqueeze(1).broadcast_to((128, NBH, NB)),
                op=OP.add)
            nc.scalar.activation(hh[:], hh[:], AF.Relu)
            e4 = sb.tile([4, NBH * NB], f32, tag="e4")
            for n in range(4):
                sl = slice(n * 512, (n + 1) * 512)
                psI = int_ps.tile([4, 512], f32, tag="int")
                nc.tensor.matmul(psI[:], lhsT=w2bd[:], rhs=hh[:, sl],
                                 start=True, stop=True)
                nc.scalar.activation(e4[:, sl], psI[:], AF.Exp,
                                     bias=nb2i[:], scale=-1.0)
                nc.scalar.activation(e4[:, sl], e4[:, sl], AF.Identity,
                                     bias=one4[:])
                nc.vector.reciprocal(e4[:, sl], e4[:, sl])
                nc.vector.tensor_tensor(e4[:, sl], e4[:, sl], qk[:, sl],
                                        op=OP.mult)
            if g < 3:
                # scatter head rows into estage3 [(32 i) p, 64 j @ hh*64]
                for cc in range(4):
                    nc.scalar.dma_start(
                        estage3[:, (4 * g + cc) * NB:(4 * g + cc + 1) * NB],
                        e4[cc:cc + 1, :])
                # incremental (hh,j)->(j,hh) transpose for this grp's columns
                nc.scalar.copy(
                    estage4[:].rearrange("p (j hh) -> p hh j", j=NB, hh=16)
                    [:, 4 * g:4 * g + 4, :],
                    estage3[:, 4 * g * NB:(4 * g + 4) * NB]
                    .rearrange("p (hh j) -> p hh j", hh=4, j=NB))
                # hh-sliced fold for this grp (hidden under the stream)
                for rr in range(NROW):
                    nc.gpsimd.dma_start(
                        foldedT[16 * rr:16 * rr + 16, :]
                        .rearrange("p (jr hh) -> p jr hh", jr=16, hh=16)
                        [:, :, 4 * g:4 * g + 4],
                        estage4[4 * rr:4 * rr + 4, :]
                        .rearrange("p (jq jr hh) -> p jq jr hh",
                                   jq=4, jr=16, hh=16)[:, :, :, 4 * g:4 * g + 4])
            else:
                # last grp: e4 -> foldedT directly (tail path, idle DMA device)
                for cc in range(4):
                    hhg = 4 * g + cc
                    eng = nc.sync if cc % 2 == 0 else nc.scalar
                    eng.dma_start(
                        foldedT[:, :]
                        .rearrange("p (jr hh) -> p jr hh", jr=16, hh=16)
                        [:, :, hhg:hhg + 1],
                        e4[cc:cc + 1, :]
                        .rearrange("o (p jr) -> o p jr", p=128, jr=16)
                        .unsqueeze(3))

        # ---- emit program ----
        for h in range(H):
            pool_dve(ks, h, bigk, kPack, "k")
        nc.sync.dma_start(cc_in[:], kPack[:])
        nc.gpsimd.collective_compute(
            "AllGather", mybir.AluOpType.bypass,
            replica_groups=[[0, 1], [2, 3], [4, 5], [6, 7]],
            ins=[cc_in.opt()], outs=[cc_out.opt()])
        def q_grp(g):
            paths = [pool_dve, pool_pe, pool_pe, pool_dve] if g in (0, 3) \
                else [pool_pe, pool_dve, pool_pe, pool_pe]
            for cc in range(4):
                paths[cc](qs, 4 * g + cc, bigq, qTall, "q")

        def grp_compute(g):
            mlp(qTall, g * 128, (g + 1) * 128, qimp[0:1, g * 128:(g + 1) * 128])
            mlp(kTall, g * 256, (g + 1) * 256, kimp[0:1, g * 256:(g + 1) * 256])
            sigma_inplace(qimp[0:1, g * 128:(g + 1) * 128])
            sigma_inplace(kimp[0:1, g * 256:(g + 1) * 256])
            interact_grp(g)

        q_grp(0)
        q_grp(1)
        nc.sync.dma_start(ktmp[:, 0:H * NBH], cc_out[0:128])
        nc.sync.dma_start(ktmp[:, H * NBH:], cc_out[128:256])
        # interleave on Pool (idle engine; ACT would head-block pool copies)
        nc.gpsimd.tensor_copy(
            kTall[:].rearrange("p (h c j) -> p c h j", h=H, c=2, j=NBH),
            ktmp[:].rearrange("p (c h j) -> p c h j", c=2, h=H, j=NBH))
        grp_compute(0)
        q_grp(2)
        grp_compute(1)
        grp_compute(2)
        q_grp(3)
        grp_compute(3)

        # ---- top-k: two-phase exact fp32 bisection (transposed layout) ----
        thr = persist.tile([128, 1], f32, tag="thr")
        pr = persist.tile([128, 1], f32, tag="pr")
        lo = persist.tile([128, 1], f32, tag="lo")
        ge = persist.tile([128, 256], f32, tag="ge")
        cntp = persist.tile([128, 1], f32, tag="cntp")

        def bisect(vals, nit):
            nc.vector.memset(thr[:], 0.5)
            s = 0.25
            for _ in range(nit):
                psC = small_ps.tile([128, 1], f32, tag="mlp")
                nc.vector.tensor_tensor(
                    ge[:], vals[:], thr[:].broadcast_to((128, 256)),
                    op=OP.is_ge)
                nc.vector.tensor_reduce(cntp[:], ge[:], axis=AX.X, op=OP.add)
                nc.tensor.matmul(psC[:], lhsT=blk16[:], rhs=cntp[:],
                                 start=True, stop=True)
                # pr = (cnt >= K) * 2s ; thr += pr - s
                nc.vector.tensor_scalar(pr[:], psC[:], float(KSEL), 2.0 * s,
                                        op0=OP.is_ge, op1=OP.mult)
                nc.vector.scalar_tensor_tensor(thr[:], pr[:], -s, thr[:],
                                               op0=OP.add, op1=OP.add)
                s *= 0.5
            return s

        s1 = bisect(foldedT, NIT1)
        nc.vector.tensor_scalar_add(lo[:], thr[:], -2.0 * s1)
        nc.vector.tensor_scalar(foldedT2[:], foldedT[:], lo[:],
                                float(2 ** NIT1), op0=OP.subtract, op1=OP.mult)
        s2 = bisect(foldedT2, NIT2)
        nc.vector.tensor_scalar_add(lo[:], thr[:], -2.0 * s2)

        mask = persist.tile([128, 256], u8, tag="mask")
        nc.vector.tensor_scalar(mask[:], foldedT2[:], lo[:], None, op0=OP.is_ge)
        nc.sync.dma_start(
            y[:].rearrange("r (a jq jr4) (jrm hh) -> (r a jq) jr4 jrm hh",
                           a=4, jq=4, jr4=4, jrm=4, hh=16),
            mask[:].rearrange("p (jr4 jrm hh) -> p jr4 jrm hh",
                              jr4=4, jrm=4, hh=16))

    nc.compile()
    return nc


def _constants(w_imp1, b_imp1, w_imp2, b_imp2, w_imp3, b_imp3,
               w_int1, b_int1, w_int2, b_int2):
    f = np.float32
    blkS = np.zeros((128, NBH), f)
    blkS[np.arange(128), np.arange(128) // 4] = 1.0 / 128.0
    blkq = np.zeros((128, 32), f)
    blkq[np.arange(128), np.arange(128) // 4] = 1.0
    blk16 = np.zeros((128, 128), f)
    blk16[np.arange(128)[:, None] // 16 == np.arange(128)[None, :] // 16] = 1.0
    consts = {
        "c_blkS": blkS,
        "c_blkq": blkq,
        "c_id32": (np.eye(32, dtype=f) / 128.0).astype(f),
        "c_blk16": blk16,
        "c_w1": np.ascontiguousarray(w_imp1, f),
        "c_b1": np.ascontiguousarray(np.asarray(b_imp1, f).reshape(HID1, 1)),
        "c_w2": np.ascontiguousarray(w_imp2, f),
        "c_b2": np.ascontiguousarray(np.asarray(b_imp2, f).reshape(HID2, 1)),
        "c_w3": np.ascontiguousarray(w_imp3, f),
        "c_nb3": np.ascontiguousarray(-np.asarray(b_imp3, f).reshape(1, 1)),
        "c_w1a": np.ascontiguousarray(np.asarray(w_int1, f)[:D]),
        "c_w1b": np.ascontiguousarray(np.asarray(w_int1, f)[D:]),
        "c_b1i": np.ascontiguousarray(
            np.tile(np.asarray(b_int1, f).reshape(HID1, 1), (4, 1))),
        "c_nb2i": np.ascontiguousarray(
            np.tile(-np.asarray(b_int2, f).reshape(1, 1), (4, 1))),
        "c_one4": np.ones((4, 1), f),
    }
    w2bd = np.zeros((128, 4), f)
    for c in range(4):
        w2bd[32 * c:32 * c + 32, c] = np.asarray(w_int2, f)[:, 0]
    consts["c_w2bd"] = w2bd
    return consts


def _in_maps(q, k, w_imp1, b_imp1, w_imp2, b_imp2, w_imp3, b_imp3,
             w_int1, b_int1, w_int2, b_int2):
    q = np.asarray(q, np.float32)
    k = np.asarray(k, np.float32)
    consts = _constants(w_imp1, b_imp1, w_imp2, b_imp2, w_imp3, b_imp3,
                        w_int1, b_int1, w_int2, b_int2)
    in_maps = []
    for c in range(NCORES):
        b, p = c // 2, c % 2
        m = {"qs": q[b, p * SH:(p + 1) * SH],
             "ks": k[b, p * SH:(p + 1) * SH]}
        m.update(consts)
        in_maps.append(m)
    return in_maps


def kernel(q, k, **w):
    from concourse.bass_utils import run_bass_kernel_spmd

    in_maps = _in_maps(q, k, **w)

    if "nc" not in _nc_cache:
        _nc_cache["nc"] = _build_nc()
    res = run_bass_kernel_spmd(_nc_cache["nc"], in_maps,
                               core_ids=list(range(NCORES)))
    out = np.empty((B, H, NB, NB), np.uint8)
    for c in range(NCORES):
        b, p = c // 2, c % 2
        out[b, p * 8:(p + 1) * 8] = res.results[c]["y"]
    return out > 0
